# revision 9
# baseline (speedup 1.0000x reference)
"""Trainium2 Bass kernel for nn_ComprehensiveLoss (BCE+Dice+FocalTversky+
Boundary+clDice+Hausdorff) on [32,1,512,512] inputs.

Strategy: pure data parallel over batch — 4 images per core as two
column-interleaved image pairs. All morphology (soft-skeleton, erosion
distance transforms, boundary) runs fused in SBUF in bf16. Cross-partition
halo rows are produced by TensorE shift-matmuls into PSUM and copied back by
the Scalar engine, so the Vector engine never waits on DMA. The two pairs'
instruction streams are interleaved so one pair's DVE work hides the other
pair's halo latency. Each core emits per-partition partial sums; the final
scalar ratios are combined on the host.

Layout: each image pair is column-interleaved (position 2c+img) so every
1-column stencil shift stays 4-byte aligned (keeps DVE 2x mode). Partition p
holds rows 4p..4p+3 of both images plus 2 halo rows (tile rows 0 and 5).
"""
import numpy as np
import concourse.bacc as bacc
import concourse.mybir as mybir
from concourse.tile import TileContext
from concourse.bass_utils import run_bass_kernel_spmd

F32 = mybir.dt.float32
BF16 = mybir.dt.bfloat16
I32 = mybir.dt.int32
OP = mybir.AluOpType
AF = mybir.ActivationFunctionType
AX = mybir.AxisListType

P = 128
NCORES = 8
H = W = 512
C2 = 2 * W           # interleaved row width
RPP = 4              # owned rows per partition
K_SKEL = 10          # reference soft_skeleton iters

# stats column map (per pair); ACT-accumulated stats get one column/image
C_SP = 0      # +i: sum softplus(pred)
C_P = 2       # +i: sum sigmoid(pred)
C_T = 4       # +i: sum t
C_PT = 6      # sum pred*t
C_PROBT = 7   # sum prob*t
C_MASK = 8    # sum (pred<=0)
C_BSP = 9     # sum boundary*softplus
C_BPT = 10    # sum boundary*pred*t
C_SPT = 11    # sum u_pred*t
C_SPS = 12    # sum u_pred
C_STP = 13    # sum u_true*prob
C_STS = 14    # sum u_true
C_DTP = 15    # sum dist_p*t
C_DTT = 16    # sum dist_t
C_DTTM = 17   # sum dist_t*mask_p
STC = 18


def _img(view, i):
    """image-i sub-view of an interleaved [...,1024] view"""
    return view.rearrange("p r (c two) -> p r c two", two=2)[:, :, :, i]


def _epair(v, a, b):
    """[P,4,1024] view -> positions {a,a+1,b,b+1} as [P,4,2,2] (b>a, even)"""
    g = v.rearrange("p r (g c) -> p r g c", c=2)
    return g[:, :, a // 2:b // 2 + 1:(b - a) // 2, :]


class _Pair:
    """Per-pair tiles + op emitters. All emitters are fine-grained so the
    build loop can interleave the two pairs' instruction streams."""

    def __init__(self, nc, pool, psum_pool, shm, pair):
        self.nc = nc
        self.shm = shm  # [128, 4, 128] bf16 shift matrices
        s = f"_{pair}"
        # 6-row tiles (owned rows 1..4, halo rows 0 and 5)
        self.T = pool.tile([P, 6, C2], BF16, name="T" + s, tag="T" + s)
        self.PR = pool.tile([P, 6, C2], BF16, name="PR" + s, tag="PR" + s)
        self.MK = pool.tile([P, 6, C2], BF16, name="MK" + s, tag="MK" + s)
        # E-slots double as phase-1 planar staging (PRD / TB) via shared tags
        self.PRD = pool.tile([P, 2, RPP, W], BF16, name="PRD" + s, tag="E1" + s)
        self.TB = pool.tile([P, 2, RPP, W], BF16, name="TB" + s, tag="E2" + s)
        # 4-row scratch
        self.A = pool.tile([P, RPP, C2], BF16, name="A" + s, tag="A" + s)
        self.B = pool.tile([P, RPP, C2], BF16, name="B" + s, tag="B" + s)
        self.C = pool.tile([P, RPP, C2], BF16, name="C" + s, tag="C" + s)
        # LU: logits in phase1/boundary, then u-product ping-pong slot 0
        self.LU = pool.tile([P, RPP, C2], BF16, name="LU" + s, tag="LU" + s)
        # SP: softplus image through boundary, then u ping-pong slot 1
        self.SP = pool.tile([P, RPP, C2], BF16, name="SP" + s, tag="SP" + s)
        self.ST = pool.tile([P, STC], F32, name="ST" + s, tag="ST" + s)
        self.psum = psum_pool
        self.pool = pool
        self.s = s
        self.E1 = None
        self.E2 = None

    def make_e_tiles(self):
        # allocated after PRD/TB are dead; same memory via shared tags
        self.E1 = self.pool.tile([P, 6, C2], BF16, name="E1t" + self.s,
                                 tag="E1" + self.s)
        self.E2 = self.pool.tile([P, 6, C2], BF16, name="E2t" + self.s,
                                 tag="E2" + self.s)

    # ---- halo fill via TensorE + ScalarE (replaces DMA refresh) ----
    def halo(self, X):
        """fill halo rows of 6-row tile X: row0 <- below-neighbor row4
        (clamp at p=0 to own row1), row5 <- above-neighbor row1 (clamp at
        p=127 to own row4)."""
        nc, shm = self.nc, self.shm
        pp = self.psum.tile([P, 2, W], F32, name="pp" + self.s,
                            tag="pp" + self.s)
        for cc in (0, W):
            top = pp[:, 0, :] if cc == 0 else pp[:, 1, :]
            nc.tensor.matmul(top, shm[:, 0, :], X[:, 4, cc:cc + W],
                             start=True, stop=False)
            nc.tensor.matmul(top, shm[:, 2, :], X[:, 1, cc:cc + W],
                             start=False, stop=True)
        pb = self.psum.tile([P, 2, W], F32, name="pb" + self.s,
                            tag="pb" + self.s)
        for cc in (0, W):
            bot = pb[:, 0, :] if cc == 0 else pb[:, 1, :]
            nc.tensor.matmul(bot, shm[:, 1, :], X[:, 1, cc:cc + W],
                             start=True, stop=False)
            nc.tensor.matmul(bot, shm[:, 3, :], X[:, 4, cc:cc + W],
                             start=False, stop=True)
        nc.scalar.activation(out=X[:, 0, :], in_=pp.rearrange("p a b -> p (a b)"),
                             func=AF.Copy)
        nc.scalar.activation(out=X[:, 5, :], in_=pb.rearrange("p a b -> p (a b)"),
                             func=AF.Copy)

    # ---- stencil helpers (scratch discipline: A, B internal) ----
    def vpool(self, X, op, out):
        """vertical 3-tap of 6-row X -> out [P,4,1024] (uses A)"""
        nc = self.nc
        nc.vector.tensor_tensor(out=self.A[:], in0=X[:, 0:4, :],
                                in1=X[:, 2:6, :], op=op)
        nc.vector.tensor_tensor(out=out[:], in0=self.A[:],
                                in1=X[:, 1:5, :], op=op)

    def hpool(self, IN, op, out, scratch):
        """horizontal 3-tap IN [P,4,1024] -> out (clamped edges).
        scratch must differ from IN; out may alias scratch."""
        nc = self.nc
        nc.vector.tensor_tensor(out=scratch[:, :, 2:1022], in0=IN[:, :, 0:1020],
                                in1=IN[:, :, 4:1024], op=op)
        nc.vector.tensor_tensor(out=out[:, :, 2:1022], in0=scratch[:, :, 2:1022],
                                in1=IN[:, :, 2:1022], op=op)
        nc.vector.tensor_tensor(
            out=_epair(out, 0, 1022), in0=_epair(IN, 0, 1020),
            in1=_epair(IN, 2, 1022), op=op)

    def erode3(self, X, OUT):
        """3x3 min of 6-row X -> OUT [P,4,1024] (uses A, B)"""
        self.vpool(X, OP.min, self.B)
        self.hpool(self.B, OP.min, OUT, self.A)

    def soft_erode5(self, X, DST):
        """plus-shape 5-point min, 6-row X -> DST 6-row owned rows"""
        nc, A, B, C = self.nc, self.A, self.B, self.C
        Xo, Do = X[:, 1:5, :], DST[:, 1:5, :]
        nc.vector.tensor_tensor(out=A[:], in0=X[:, 0:4, :], in1=X[:, 2:6, :],
                                op=OP.min)   # m1 = min(up,down)
        nc.vector.tensor_tensor(out=B[:, :, 2:1022], in0=Xo[:, :, 0:1020],
                                in1=Xo[:, :, 4:1024], op=OP.min)  # m2
        nc.vector.tensor_tensor(out=C[:, :, 2:1022], in0=A[:, :, 2:1022],
                                in1=B[:, :, 2:1022], op=OP.min)
        nc.vector.tensor_tensor(out=Do[:, :, 2:1022], in0=C[:, :, 2:1022],
                                in1=Xo[:, :, 2:1022], op=OP.min)
        # edges: se[c0] = min(m1[c0], x[c0], x[c1]); both sides in one op
        nc.vector.tensor_tensor(out=_epair(C, 0, 1022), in0=_epair(A, 0, 1022),
                                in1=_epair(Xo, 2, 1020), op=OP.min)
        nc.vector.tensor_tensor(out=_epair(Do, 0, 1022),
                                in0=_epair(C, 0, 1022),
                                in1=_epair(Xo, 0, 1022), op=OP.min)

    # ---- skeleton pieces (emitted per-iteration by the build loop) ----
    def skel_dilate_update(self, src, dst, k, u_cur, u_nxt):
        """open = dilate3(dst) -> C; u update vs src owned rows."""
        nc, B, C = self.nc, self.B, self.C
        self.vpool(dst, OP.max, B)
        self.hpool(B, OP.max, C, self.A)   # open -> C
        # pre = open + 1 - src
        nc.vector.scalar_tensor_tensor(out=B[:], in0=C[:], scalar=1.0,
                                       in1=src[:, 1:5, :], op0=OP.add,
                                       op1=OP.subtract)
        if k == 0:
            # u0 = min(pre, 1)
            nc.vector.tensor_scalar(out=u_nxt[:], in0=B[:], scalar1=1.0,
                                    scalar2=None, op0=OP.min)
        else:
            # u = min(pre, 1) * u
            nc.vector.scalar_tensor_tensor(out=u_nxt[:], in0=B[:], scalar=1.0,
                                           in1=u_cur[:], op0=OP.min,
                                           op1=OP.mult)


def build(k_t, d_p, d_t):
    nc = bacc.Bacc("TRN2", target_bir_lowering=False, debug=False,
                   num_devices=NCORES)
    pred_d = nc.dram_tensor("pred", [2 * 2, H, W], F32, kind="ExternalInput")
    targ_d = nc.dram_tensor("target", [2 * 2, H, W], I32, kind="ExternalInput")
    out_d = nc.dram_tensor("out", [2, P, STC], F32, kind="ExternalOutput")

    with TileContext(nc) as tc, \
            tc.tile_pool(name="main", bufs=1) as pool, \
            tc.tile_pool(name="psum", bufs=1, space="PSUM") as psum_pool:
        # ---- shift matrices for halo matmuls ----
        idx = pool.tile([P, 2, P], I32, name="idx", tag="idx")
        shm = pool.tile([P, 4, P], BF16, name="shm", tag="shm")
        nc.gpsimd.iota(idx[:, 0, :], pattern=[[1, P]], base=0,
                       channel_multiplier=-1)   # f - p
        nc.gpsimd.iota(idx[:, 1, :], pattern=[[1, P]], base=0,
                       channel_multiplier=1)    # f + p
        for j, (sl, val) in enumerate(((0, 1), (0, -1), (1, 0), (1, 254))):
            nc.vector.tensor_scalar(out=shm[:, j, :], in0=idx[:, sl, :],
                                    scalar1=val, scalar2=None, op0=OP.is_equal)

        bld = [_Pair(nc, pool, psum_pool, shm, p) for p in range(2)]

        # ---- input DMA (planar, with dtype cast) ----
        for p, b in enumerate(bld):
            pv = pred_d[2 * p:2 * p + 2].rearrange("i (p r) c -> p i r c", p=P)
            nc.gpsimd.dma_start(out=b.PRD[:], in_=pv)       # f32 -> bf16
            tv = targ_d[2 * p:2 * p + 2].rearrange("i (p r) c -> p i r c", p=P)
            nc.gpsimd.dma_start(out=b.TB[:], in_=tv)        # i32 -> bf16

        # ---- phase 1: pointwise stats + interleaved images (ACT-heavy) ----
        for b in bld:                     # copies first (Copy table)
            To = b.T[:, 1:5, :]
            for i in range(2):
                nc.scalar.activation(out=_img(To, i), in_=b.TB[:, i],
                                     func=AF.Copy,
                                     accum_out=b.ST[:, C_T + i:C_T + i + 1])
            for i in range(2):
                nc.scalar.activation(out=_img(b.LU[:], i), in_=b.PRD[:, i],
                                     func=AF.Copy)
        for b in bld:
            PRo = b.PR[:, 1:5, :]
            for i in range(2):
                nc.scalar.activation(out=_img(PRo, i), in_=b.PRD[:, i],
                                     func=AF.Sigmoid,
                                     accum_out=b.ST[:, C_P + i:C_P + i + 1])
            # softplus(x) = -ln(sigmoid(-x)); SP holds the NEGATED image
            # (host flips signs of C_SP / C_BSP)
            for i in range(2):
                nc.scalar.activation(out=_img(b.C[:], i), in_=b.PRD[:, i],
                                     func=AF.Sigmoid, scale=-1.0)
        for b in bld:
            for i in range(2):
                nc.scalar.activation(out=_img(b.SP[:], i), in_=_img(b.C[:], i),
                                     func=AF.Ln,
                                     accum_out=b.ST[:, C_SP + i:C_SP + i + 1])
        for b in bld:
            To = b.T[:, 1:5, :]
            # sum(pred*t), sum(prob*t), mask + sum(mask); scratch A/B
            nc.vector.scalar_tensor_tensor(
                out=b.A[:], in0=b.LU[:], scalar=1.0, in1=To,
                op0=OP.mult, op1=OP.mult,
                accum_out=b.ST[:, C_PT:C_PT + 1])
            nc.vector.scalar_tensor_tensor(
                out=b.B[:], in0=b.PR[:, 1:5, :], scalar=1.0, in1=To,
                op0=OP.mult, op1=OP.mult,
                accum_out=b.ST[:, C_PROBT:C_PROBT + 1])
            nc.vector.tensor_scalar(out=b.MK[:, 1:5, :], in0=b.LU[:],
                                    scalar1=0.0, scalar2=1.0, op0=OP.is_le,
                                    op1=OP.mult,
                                    accum_out=b.ST[:, C_MASK:C_MASK + 1])
        for b in bld:
            b.halo(b.T)
            b.halo(b.PR)
            b.halo(b.MK)

        # ---- boundary loss sums ----
        for b in bld:
            nc_, To = b.nc, b.T[:, 1:5, :]
            b.make_e_tiles()  # PRD/TB dead from here (tag-shared memory)
            b.vpool(b.T, OP.max, b.B)
            b.hpool(b.B, OP.max, b.C, b.A)        # dilate -> C
            b.vpool(b.T, OP.min, b.B)
            b.hpool(b.B, OP.min, b.E1[:, 1:5, :], b.A)  # erode -> E1 owned
            nc_.vector.tensor_tensor(out=b.B[:], in0=b.C[:],
                                     in1=b.E1[:, 1:5, :],
                                     op=OP.subtract)          # b -> B
            nc_.vector.tensor_tensor(out=b.C[:], in0=b.B[:], in1=To,
                                     op=OP.mult)              # b*t -> C
            nc_.vector.scalar_tensor_tensor(
                out=b.A[:], in0=b.B[:], scalar=1.0, in1=b.SP[:],
                op0=OP.mult, op1=OP.mult, accum_out=b.ST[:, C_BSP:C_BSP + 1])
            nc_.vector.scalar_tensor_tensor(
                out=b.A[:], in0=b.C[:], scalar=1.0, in1=b.LU[:],
                op0=OP.mult, op1=OP.mult, accum_out=b.ST[:, C_BPT:C_BPT + 1])

        # ---- skeletons (pair-interleaved per iteration) ----
        def run_skeleton(src0_of, iters, w_of, col_prod, col_sum):
            srcs = {b: src0_of(b) for b in bld}
            ucur = {b: None for b in bld}
            for k in range(iters + 1):
                dsts = {b: (b.E1 if k % 2 == 0 else b.E2) for b in bld}
                for b in bld:
                    b.soft_erode5(srcs[b], dsts[b])
                for b in bld:
                    b.halo(dsts[b])
                for b in bld:
                    unxt = b.LU if k % 2 == 0 else b.SP
                    b.skel_dilate_update(srcs[b], dsts[b], k, ucur[b], unxt)
                    ucur[b] = unxt
                    srcs[b] = dsts[b]
            for b in bld:
                u = ucur[b]
                nc.vector.scalar_tensor_tensor(
                    out=b.A[:], in0=u[:], scalar=1.0, in1=w_of(b),
                    op0=OP.mult, op1=OP.mult,
                    accum_out=b.ST[:, col_prod:col_prod + 1])
                nc.vector.reduce_sum(out=b.ST[:, col_sum:col_sum + 1],
                                     in_=u[:], axis=AX.XY)

        run_skeleton(lambda b: b.PR, K_SKEL, lambda b: b.T[:, 1:5, :],
                     C_SPT, C_SPS)
        run_skeleton(lambda b: b.T, k_t, lambda b: b.PR[:, 1:5, :],
                     C_STP, C_STS)

        # ---- distance transforms (pair-interleaved per iteration) ----
        def run_dt(m0_of, iters, finalize):
            srcs = {b: m0_of(b) for b in bld}
            accs = {b: None for b in bld}
            for d in range(1, iters + 1):
                dsts = {b: (b.E1 if d % 2 == 1 else b.E2) for b in bld}
                for b in bld:
                    b.erode3(srcs[b], dsts[b][:, 1:5, :])
                if d < iters:
                    for b in bld:
                        b.halo(dsts[b])
                for b in bld:
                    prev = (m0_of(b)[:, 1:5, :] if accs[b] is None
                            else accs[b][:])
                    acc_n = b.C if d % 2 == 1 else b.LU
                    nc.vector.tensor_add(out=acc_n[:], in0=prev,
                                         in1=dsts[b][:, 1:5, :])
                    accs[b] = acc_n
                    srcs[b] = dsts[b]
            for b in bld:
                final = (m0_of(b)[:, 1:5, :] if accs[b] is None
                         else accs[b][:])
                finalize(b, final)

        # DT of pred mask (MK), weighted by t
        def fin_p(b, final):
            nc.vector.scalar_tensor_tensor(
                out=b.B[:], in0=final, scalar=1.0, in1=b.T[:, 1:5, :],
                op0=OP.mult, op1=OP.mult, accum_out=b.ST[:, C_DTP:C_DTP + 1])
        run_dt(lambda b: b.MK, d_p, fin_p)

        # DT of (1 - t): overwrite T with its complement; weight = 1 - mask_p
        for b in bld:
            nc.vector.tensor_scalar(out=b.T[:, 1:5, :], in0=b.T[:, 1:5, :],
                                    scalar1=-1.0, scalar2=1.0, op0=OP.mult,
                                    op1=OP.add)
        for b in bld:
            b.halo(b.T)

        def fin_t(b, final):
            nc.vector.reduce_sum(out=b.ST[:, C_DTT:C_DTT + 1], in_=final,
                                 axis=AX.XY)
            nc.vector.scalar_tensor_tensor(
                out=b.B[:], in0=final, scalar=1.0, in1=b.MK[:, 1:5, :],
                op0=OP.mult, op1=OP.mult,
                accum_out=b.ST[:, C_DTTM:C_DTTM + 1])
        run_dt(lambda b: b.T, d_t, fin_t)

        for p, b in enumerate(bld):
            nc.sync.dma_start(out=out_d[p], in_=b.ST[:])
    nc.compile()
    return nc


# ---------------- host side ----------------
_cache = {}


def _bin_soft_erode(e):
    v = e & np.roll(e, 1, 1) & np.roll(e, -1, 1)
    v[:, 0] = e[:, 0] & e[:, 1]
    v[:, -1] = e[:, -1] & e[:, -2]
    h = e & np.roll(e, 1, 2) & np.roll(e, -1, 2)
    h[:, :, 0] = e[:, :, 0] & e[:, :, 1]
    h[:, :, -1] = e[:, :, -1] & e[:, :, -2]
    return v | h


def _bin_erode3(e):
    v = e & np.roll(e, 1, 1) & np.roll(e, -1, 1)
    v[:, 0] = e[:, 0] & e[:, 1]
    v[:, -1] = e[:, -1] & e[:, -2]
    h = v & np.roll(v, 1, 2) & np.roll(v, -1, 2)
    h[:, :, 0] = v[:, :, 0] & v[:, :, 1]
    h[:, :, -1] = v[:, :, -1] & v[:, :, -2]
    return h


def _needed_iters(mask, limit, erode_fn):
    """number of erosions until empty (capped)"""
    e, n = mask, 0
    while n < limit:
        e = erode_fn(e)
        if not e.any():
            break
        n += 1
    return n


def kernel(pred, target):
    pred = np.ascontiguousarray(np.asarray(pred), dtype=np.float32)
    target = np.ascontiguousarray(np.asarray(target), dtype=np.int32)
    B = pred.shape[0]
    p3 = pred.reshape(B, H, W)
    t3 = target.reshape(B, H, W)

    tb = t3 != 0
    k_t = _needed_iters(_bin_soft_erode(tb), K_SKEL - 1, _bin_soft_erode) + 1
    k_t = min(k_t, K_SKEL)
    d_p = _needed_iters(p3 <= 0.0, 19, _bin_erode3)
    d_t = _needed_iters(~tb, 19, _bin_erode3)

    key = (k_t, d_p, d_t)
    if key not in _cache:
        _cache[key] = build(*key)
    nc = _cache[key]

    in_maps = [
        {"pred": p3[4 * c:4 * c + 4], "target": t3[4 * c:4 * c + 4]}
        for c in range(NCORES)
    ]
    res = run_bass_kernel_spmd(nc, in_maps, core_ids=list(range(NCORES)))
    st = np.stack([r["out"] for r in res.results])  # [8, 2, 128, STC]
    s = st.sum(axis=(0, 1, 2), dtype=np.float64)    # summed stats

    N = float(pred.size)
    smooth, eps, hsm = 1.0, 1.0, 1e-6
    sum_sp = -(s[C_SP] + s[C_SP + 1])
    sum_pt = s[C_PT]
    sum_p = s[C_P] + s[C_P + 1]
    inter = s[C_PROBT]
    sum_t = s[C_T] + s[C_T + 1]
    loss_bce = (sum_sp - sum_pt) / N
    loss_dice = 1.0 - (2.0 * inter + smooth) / (sum_p + sum_t + smooth)
    fp = sum_p - inter
    fn = sum_t - inter
    tversky = (inter + smooth) / (inter + 0.3 * fp + 0.7 * fn + smooth)
    loss_ft = (1.0 - tversky) ** 1.33
    loss_boundary = loss_bce + 3.0 * (-s[C_BSP] - s[C_BPT]) / N
    tprec = ((sum_t - s[C_SPT]) + eps) / ((N - s[C_SPS]) + eps)
    tsens = ((sum_p - s[C_STP]) + eps) / ((N - s[C_STS]) + eps)
    loss_cldice = 1.0 - 2.0 * tprec * tsens / (tprec + tsens)
    dtp = s[C_DTP]
    dtt = s[C_DTT] - s[C_DTTM]
    n_pb = N - s[C_MASK]
    hd_fwd = (dtp + hsm) / (sum_t + hsm)
    hd_bwd = (dtt + hsm) / (n_pb + hsm)
    loss_hd = 0.5 * (hd_fwd + hd_bwd)
    total = (0.2 * loss_bce + 0.2 * loss_dice + 0.2 * loss_cldice
             + 0.1 * loss_hd + 0.1 * loss_boundary + 0.2 * loss_ft)
    return np.float32(total)


# revision 10
# speedup vs baseline: 1.0011x; 1.0011x over previous
"""Trainium2 Bass kernel for nn_ComprehensiveLoss (BCE+Dice+FocalTversky+
Boundary+clDice+Hausdorff) on [32,1,512,512] inputs.

Strategy: pure data parallel over batch — 4 images per core as two
column-interleaved image pairs. All morphology (soft-skeleton, erosion
distance transforms, boundary) runs fused in SBUF in bf16. Cross-partition
halo rows are produced by TensorE shift-matmuls into PSUM and copied back by
the Scalar engine, so the Vector engine never waits on DMA. The two pairs'
instruction streams are interleaved so one pair's DVE work hides the other
pair's halo latency. Each core emits per-partition partial sums; the final
scalar ratios are combined on the host.

Layout: each image pair is column-interleaved (position 2c+img) so every
1-column stencil shift stays 4-byte aligned (keeps DVE 2x mode). Partition p
holds rows 4p..4p+3 of both images plus 2 halo rows (tile rows 0 and 5).
"""
import numpy as np
import concourse.bacc as bacc
import concourse.mybir as mybir
from concourse.tile import TileContext
from concourse.bass_utils import run_bass_kernel_spmd

F32 = mybir.dt.float32
BF16 = mybir.dt.bfloat16
I32 = mybir.dt.int32
OP = mybir.AluOpType
AF = mybir.ActivationFunctionType
AX = mybir.AxisListType

P = 128
NCORES = 8
H = W = 512
C2 = 2 * W           # interleaved row width
RPP = 4              # owned rows per partition
K_SKEL = 10          # reference soft_skeleton iters

# stats column map (per pair); ACT-accumulated stats get one column/image
C_SP = 0      # +i: sum softplus(pred)
C_P = 2       # +i: sum sigmoid(pred)
C_T = 4       # +i: sum t
C_PT = 6      # sum pred*t
C_PROBT = 7   # sum prob*t
C_MASK = 8    # sum (pred<=0)
C_BSP = 9     # sum boundary*softplus
C_BPT = 10    # sum boundary*pred*t
C_SPT = 11    # sum u_pred*t
C_SPS = 12    # sum u_pred
C_STP = 13    # sum u_true*prob
C_STS = 14    # sum u_true
C_DTP = 15    # sum dist_p*t
C_DTT = 16    # sum dist_t
C_DTTM = 17   # sum dist_t*mask_p
STC = 18


def _img(view, i):
    """image-i sub-view of an interleaved [...,1024] view"""
    return view.rearrange("p r (c two) -> p r c two", two=2)[:, :, :, i]


def _epair(v, a, b):
    """[P,4,1024] view -> positions {a,a+1,b,b+1} as [P,4,2,2] (b>a, even)"""
    g = v.rearrange("p r (g c) -> p r g c", c=2)
    return g[:, :, a // 2:b // 2 + 1:(b - a) // 2, :]


class _Pair:
    """Per-pair tiles + op emitters. All emitters are fine-grained so the
    build loop can interleave the two pairs' instruction streams."""

    def __init__(self, nc, pool, psum_pool, shm, pair):
        self.nc = nc
        self.shm = shm  # [128, 4, 128] bf16 shift matrices
        s = f"_{pair}"
        # 6-row tiles (owned rows 1..4, halo rows 0 and 5)
        self.T = pool.tile([P, 6, C2], BF16, name="T" + s, tag="T" + s)
        self.PR = pool.tile([P, 6, C2], BF16, name="PR" + s, tag="PR" + s)
        self.MK = pool.tile([P, 6, C2], BF16, name="MK" + s, tag="MK" + s)
        # E-slots double as phase-1 planar staging (PRD / TB) via shared tags
        self.PRD = pool.tile([P, 2, RPP, W], BF16, name="PRD" + s, tag="E1" + s)
        self.TB = pool.tile([P, 2, RPP, W], BF16, name="TB" + s, tag="E2" + s)
        # 4-row scratch
        self.A = pool.tile([P, RPP, C2], BF16, name="A" + s, tag="A" + s)
        self.B = pool.tile([P, RPP, C2], BF16, name="B" + s, tag="B" + s)
        self.C = pool.tile([P, RPP, C2], BF16, name="C" + s, tag="C" + s)
        # LU: logits in phase1/boundary, then u-product ping-pong slot 0
        self.LU = pool.tile([P, RPP, C2], BF16, name="LU" + s, tag="LU" + s)
        # SP: softplus image through boundary, then u ping-pong slot 1
        self.SP = pool.tile([P, RPP, C2], BF16, name="SP" + s, tag="SP" + s)
        self.ST = pool.tile([P, STC], F32, name="ST" + s, tag="ST" + s)
        self.psum = psum_pool
        self.pool = pool
        self.s = s
        self.E1 = None
        self.E2 = None

    def make_e_tiles(self):
        # allocated after PRD/TB are dead; same memory via shared tags
        self.E1 = self.pool.tile([P, 6, C2], BF16, name="E1t" + self.s,
                                 tag="E1" + self.s)
        self.E2 = self.pool.tile([P, 6, C2], BF16, name="E2t" + self.s,
                                 tag="E2" + self.s)

    # ---- halo fill via TensorE + ScalarE (replaces DMA refresh) ----
    def halo(self, X):
        """fill halo rows of 6-row tile X: row0 <- below-neighbor row4
        (clamp at p=0 to own row1), row5 <- above-neighbor row1 (clamp at
        p=127 to own row4)."""
        nc, shm = self.nc, self.shm
        pp = self.psum.tile([P, 2, W], F32, name="pp" + self.s,
                            tag="pp" + self.s)
        for cc in (0, W):
            top = pp[:, 0, :] if cc == 0 else pp[:, 1, :]
            nc.tensor.matmul(top, shm[:, 0, :], X[:, 4, cc:cc + W],
                             start=True, stop=False)
            nc.tensor.matmul(top, shm[:, 2, :], X[:, 1, cc:cc + W],
                             start=False, stop=True)
        pb = self.psum.tile([P, 2, W], F32, name="pb" + self.s,
                            tag="pb" + self.s)
        for cc in (0, W):
            bot = pb[:, 0, :] if cc == 0 else pb[:, 1, :]
            nc.tensor.matmul(bot, shm[:, 1, :], X[:, 1, cc:cc + W],
                             start=True, stop=False)
            nc.tensor.matmul(bot, shm[:, 3, :], X[:, 4, cc:cc + W],
                             start=False, stop=True)
        nc.scalar.activation(out=X[:, 0, :], in_=pp.rearrange("p a b -> p (a b)"),
                             func=AF.Copy)
        nc.scalar.activation(out=X[:, 5, :], in_=pb.rearrange("p a b -> p (a b)"),
                             func=AF.Copy)

    # ---- stencil helpers (scratch discipline: A, B internal) ----
    def vpool(self, X, op, out):
        """vertical 3-tap of 6-row X -> out [P,4,1024] (uses A)"""
        nc = self.nc
        nc.vector.tensor_tensor(out=self.A[:], in0=X[:, 0:4, :],
                                in1=X[:, 2:6, :], op=op)
        nc.vector.tensor_tensor(out=out[:], in0=self.A[:],
                                in1=X[:, 1:5, :], op=op)

    def hpool(self, IN, op, out, scratch):
        """horizontal 3-tap IN [P,4,1024] -> out (clamped edges).
        scratch must differ from IN; out may alias scratch."""
        nc = self.nc
        nc.vector.tensor_tensor(out=scratch[:, :, 2:1022], in0=IN[:, :, 0:1020],
                                in1=IN[:, :, 4:1024], op=op)
        nc.vector.tensor_tensor(out=out[:, :, 2:1022], in0=scratch[:, :, 2:1022],
                                in1=IN[:, :, 2:1022], op=op)
        nc.vector.tensor_tensor(
            out=_epair(out, 0, 1022), in0=_epair(IN, 0, 1020),
            in1=_epair(IN, 2, 1022), op=op)

    def erode3(self, X, OUT):
        """3x3 min of 6-row X -> OUT [P,4,1024] (uses A, B)"""
        self.vpool(X, OP.min, self.B)
        self.hpool(self.B, OP.min, OUT, self.A)

    def soft_erode5(self, X, DST):
        """plus-shape 5-point min, 6-row X -> DST 6-row owned rows"""
        nc, A, B, C = self.nc, self.A, self.B, self.C
        Xo, Do = X[:, 1:5, :], DST[:, 1:5, :]
        nc.vector.tensor_tensor(out=A[:], in0=X[:, 0:4, :], in1=X[:, 2:6, :],
                                op=OP.min)   # m1 = min(up,down)
        nc.vector.tensor_tensor(out=B[:, :, 2:1022], in0=Xo[:, :, 0:1020],
                                in1=Xo[:, :, 4:1024], op=OP.min)  # m2
        nc.vector.tensor_tensor(out=C[:, :, 2:1022], in0=A[:, :, 2:1022],
                                in1=B[:, :, 2:1022], op=OP.min)
        nc.vector.tensor_tensor(out=Do[:, :, 2:1022], in0=C[:, :, 2:1022],
                                in1=Xo[:, :, 2:1022], op=OP.min)
        # edges: se[c0] = min(m1[c0], x[c0], x[c1]); both sides in one op
        nc.vector.tensor_tensor(out=_epair(C, 0, 1022), in0=_epair(A, 0, 1022),
                                in1=_epair(Xo, 2, 1020), op=OP.min)
        nc.vector.tensor_tensor(out=_epair(Do, 0, 1022),
                                in0=_epair(C, 0, 1022),
                                in1=_epair(Xo, 0, 1022), op=OP.min)

    # ---- skeleton pieces (emitted per-iteration by the build loop) ----
    def skel_dilate_update(self, src, dst, k, u_cur, u_nxt):
        """open = dilate3(dst) -> C; u update vs src owned rows."""
        nc, B, C = self.nc, self.B, self.C
        self.vpool(dst, OP.max, B)
        self.hpool(B, OP.max, C, self.A)   # open -> C
        # pre = open + 1 - src
        nc.vector.scalar_tensor_tensor(out=B[:], in0=C[:], scalar=1.0,
                                       in1=src[:, 1:5, :], op0=OP.add,
                                       op1=OP.subtract)
        if k == 0:
            # u0 = min(pre, 1)
            nc.vector.tensor_scalar(out=u_nxt[:], in0=B[:], scalar1=1.0,
                                    scalar2=None, op0=OP.min)
        else:
            # u = min(pre, 1) * u
            nc.vector.scalar_tensor_tensor(out=u_nxt[:], in0=B[:], scalar=1.0,
                                           in1=u_cur[:], op0=OP.min,
                                           op1=OP.mult)


def build(k_t, d_p, d_t):
    nc = bacc.Bacc("TRN2", target_bir_lowering=False, debug=False,
                   num_devices=NCORES)
    pred_d = nc.dram_tensor("pred", [2 * 2, H, W], F32, kind="ExternalInput")
    targ_d = nc.dram_tensor("target", [2 * 2, H, W], I32, kind="ExternalInput")
    out_d = nc.dram_tensor("out", [2, P, STC], F32, kind="ExternalOutput")

    with TileContext(nc) as tc, \
            tc.tile_pool(name="main", bufs=1) as pool, \
            tc.tile_pool(name="psum", bufs=1, space="PSUM") as psum_pool:
        # ---- shift matrices for halo matmuls ----
        idx = pool.tile([P, 2, P], I32, name="idx", tag="idx")
        shm = pool.tile([P, 4, P], BF16, name="shm", tag="shm")
        nc.gpsimd.iota(idx[:, 0, :], pattern=[[1, P]], base=0,
                       channel_multiplier=-1)   # f - p
        nc.gpsimd.iota(idx[:, 1, :], pattern=[[1, P]], base=0,
                       channel_multiplier=1)    # f + p
        for j, (sl, val) in enumerate(((0, 1), (0, -1), (1, 0), (1, 254))):
            nc.vector.tensor_scalar(out=shm[:, j, :], in0=idx[:, sl, :],
                                    scalar1=val, scalar2=None, op0=OP.is_equal)

        bld = [_Pair(nc, pool, psum_pool, shm, p) for p in range(2)]

        # ---- input DMA (planar, with dtype cast) ----
        for p, b in enumerate(bld):
            pv = pred_d[2 * p:2 * p + 2].rearrange("i (p r) c -> p i r c", p=P)
            nc.gpsimd.dma_start(out=b.PRD[:], in_=pv)       # f32 -> bf16
            tv = targ_d[2 * p:2 * p + 2].rearrange("i (p r) c -> p i r c", p=P)
            nc.gpsimd.dma_start(out=b.TB[:], in_=tv)        # i32 -> bf16

        # ---- phase 1: pointwise stats + interleaved images (ACT-heavy) ----
        for b in bld:                     # copies first (Copy table)
            To = b.T[:, 1:5, :]
            for i in range(2):
                nc.scalar.activation(out=_img(To, i), in_=b.TB[:, i],
                                     func=AF.Copy,
                                     accum_out=b.ST[:, C_T + i:C_T + i + 1])
            for i in range(2):
                nc.scalar.activation(out=_img(b.LU[:], i), in_=b.PRD[:, i],
                                     func=AF.Copy)
        for b in bld:
            PRo = b.PR[:, 1:5, :]
            for i in range(2):
                nc.scalar.activation(out=_img(PRo, i), in_=b.PRD[:, i],
                                     func=AF.Sigmoid,
                                     accum_out=b.ST[:, C_P + i:C_P + i + 1])
            # softplus(x) = -ln(sigmoid(-x)); SP holds the NEGATED image
            # (host flips signs of C_SP / C_BSP)
            for i in range(2):
                nc.scalar.activation(out=_img(b.C[:], i), in_=b.PRD[:, i],
                                     func=AF.Sigmoid, scale=-1.0)
        for b in bld:
            for i in range(2):
                nc.scalar.activation(out=_img(b.SP[:], i), in_=_img(b.C[:], i),
                                     func=AF.Ln,
                                     accum_out=b.ST[:, C_SP + i:C_SP + i + 1])
        for b in bld:
            To = b.T[:, 1:5, :]
            # sum(pred*t), sum(prob*t), mask + sum(mask); scratch A/B
            nc.vector.scalar_tensor_tensor(
                out=b.A[:], in0=b.LU[:], scalar=1.0, in1=To,
                op0=OP.mult, op1=OP.mult,
                accum_out=b.ST[:, C_PT:C_PT + 1])
            nc.vector.scalar_tensor_tensor(
                out=b.B[:], in0=b.PR[:, 1:5, :], scalar=1.0, in1=To,
                op0=OP.mult, op1=OP.mult,
                accum_out=b.ST[:, C_PROBT:C_PROBT + 1])
            nc.vector.tensor_scalar(out=b.MK[:, 1:5, :], in0=b.LU[:],
                                    scalar1=0.0, scalar2=0.0, op0=OP.is_le,
                                    op1=OP.add,
                                    accum_out=b.ST[:, C_MASK:C_MASK + 1])
        for b in bld:
            b.halo(b.T)
            b.halo(b.PR)
            b.halo(b.MK)

        # ---- boundary loss sums ----
        for b in bld:
            nc_, To = b.nc, b.T[:, 1:5, :]
            b.make_e_tiles()  # PRD/TB dead from here (tag-shared memory)
            b.vpool(b.T, OP.max, b.B)
            b.hpool(b.B, OP.max, b.C, b.A)        # dilate -> C
            b.vpool(b.T, OP.min, b.B)
            b.hpool(b.B, OP.min, b.E1[:, 1:5, :], b.A)  # erode -> E1 owned
            nc_.vector.tensor_tensor(out=b.B[:], in0=b.C[:],
                                     in1=b.E1[:, 1:5, :],
                                     op=OP.subtract)          # b -> B
            nc_.vector.tensor_tensor(out=b.C[:], in0=b.B[:], in1=To,
                                     op=OP.mult)              # b*t -> C
            nc_.vector.scalar_tensor_tensor(
                out=b.A[:], in0=b.B[:], scalar=1.0, in1=b.SP[:],
                op0=OP.mult, op1=OP.mult, accum_out=b.ST[:, C_BSP:C_BSP + 1])
            nc_.vector.scalar_tensor_tensor(
                out=b.A[:], in0=b.C[:], scalar=1.0, in1=b.LU[:],
                op0=OP.mult, op1=OP.mult, accum_out=b.ST[:, C_BPT:C_BPT + 1])

        # ---- skeletons (pair-interleaved per iteration) ----
        def run_skeleton(src0_of, iters, w_of, col_prod, col_sum):
            srcs = {b: src0_of(b) for b in bld}
            ucur = {b: None for b in bld}
            for k in range(iters + 1):
                dsts = {b: (b.E1 if k % 2 == 0 else b.E2) for b in bld}
                for b in bld:
                    b.soft_erode5(srcs[b], dsts[b])
                for b in bld:
                    b.halo(dsts[b])
                for b in bld:
                    unxt = b.LU if k % 2 == 0 else b.SP
                    b.skel_dilate_update(srcs[b], dsts[b], k, ucur[b], unxt)
                    ucur[b] = unxt
                    srcs[b] = dsts[b]
            for b in bld:
                u = ucur[b]
                nc.vector.scalar_tensor_tensor(
                    out=b.A[:], in0=u[:], scalar=1.0, in1=w_of(b),
                    op0=OP.mult, op1=OP.mult,
                    accum_out=b.ST[:, col_prod:col_prod + 1])
                nc.vector.reduce_sum(out=b.ST[:, col_sum:col_sum + 1],
                                     in_=u[:], axis=AX.XY)

        run_skeleton(lambda b: b.PR, K_SKEL, lambda b: b.T[:, 1:5, :],
                     C_SPT, C_SPS)
        run_skeleton(lambda b: b.T, k_t, lambda b: b.PR[:, 1:5, :],
                     C_STP, C_STS)

        # ---- distance transforms (pair-interleaved per iteration) ----
        def run_dt(m0_of, iters, finalize):
            srcs = {b: m0_of(b) for b in bld}
            accs = {b: None for b in bld}
            for d in range(1, iters + 1):
                dsts = {b: (b.E1 if d % 2 == 1 else b.E2) for b in bld}
                for b in bld:
                    b.erode3(srcs[b], dsts[b][:, 1:5, :])
                if d < iters:
                    for b in bld:
                        b.halo(dsts[b])
                for b in bld:
                    prev = (m0_of(b)[:, 1:5, :] if accs[b] is None
                            else accs[b][:])
                    acc_n = b.C if d % 2 == 1 else b.LU
                    nc.vector.tensor_add(out=acc_n[:], in0=prev,
                                         in1=dsts[b][:, 1:5, :])
                    accs[b] = acc_n
                    srcs[b] = dsts[b]
            for b in bld:
                final = (m0_of(b)[:, 1:5, :] if accs[b] is None
                         else accs[b][:])
                finalize(b, final)

        # DT of pred mask (MK), weighted by t
        def fin_p(b, final):
            nc.vector.scalar_tensor_tensor(
                out=b.B[:], in0=final, scalar=1.0, in1=b.T[:, 1:5, :],
                op0=OP.mult, op1=OP.mult, accum_out=b.ST[:, C_DTP:C_DTP + 1])
        run_dt(lambda b: b.MK, d_p, fin_p)

        # DT of (1 - t): overwrite T with its complement; weight = 1 - mask_p
        for b in bld:
            nc.vector.tensor_scalar(out=b.T[:, 1:5, :], in0=b.T[:, 1:5, :],
                                    scalar1=-1.0, scalar2=1.0, op0=OP.mult,
                                    op1=OP.add)
        for b in bld:
            b.halo(b.T)

        def fin_t(b, final):
            nc.vector.reduce_sum(out=b.ST[:, C_DTT:C_DTT + 1], in_=final,
                                 axis=AX.XY)
            nc.vector.scalar_tensor_tensor(
                out=b.B[:], in0=final, scalar=1.0, in1=b.MK[:, 1:5, :],
                op0=OP.mult, op1=OP.mult,
                accum_out=b.ST[:, C_DTTM:C_DTTM + 1])
        run_dt(lambda b: b.T, d_t, fin_t)

        for p, b in enumerate(bld):
            nc.sync.dma_start(out=out_d[p], in_=b.ST[:])
    nc.compile()
    return nc


# ---------------- host side ----------------
_cache = {}


def _bin_soft_erode(e):
    v = e & np.roll(e, 1, 1) & np.roll(e, -1, 1)
    v[:, 0] = e[:, 0] & e[:, 1]
    v[:, -1] = e[:, -1] & e[:, -2]
    h = e & np.roll(e, 1, 2) & np.roll(e, -1, 2)
    h[:, :, 0] = e[:, :, 0] & e[:, :, 1]
    h[:, :, -1] = e[:, :, -1] & e[:, :, -2]
    return v | h


def _bin_erode3(e):
    v = e & np.roll(e, 1, 1) & np.roll(e, -1, 1)
    v[:, 0] = e[:, 0] & e[:, 1]
    v[:, -1] = e[:, -1] & e[:, -2]
    h = v & np.roll(v, 1, 2) & np.roll(v, -1, 2)
    h[:, :, 0] = v[:, :, 0] & v[:, :, 1]
    h[:, :, -1] = v[:, :, -1] & v[:, :, -2]
    return h


def _needed_iters(mask, limit, erode_fn):
    """number of erosions until empty (capped)"""
    e, n = mask, 0
    while n < limit:
        e = erode_fn(e)
        if not e.any():
            break
        n += 1
    return n


def kernel(pred, target):
    pred = np.ascontiguousarray(np.asarray(pred), dtype=np.float32)
    target = np.ascontiguousarray(np.asarray(target), dtype=np.int32)
    B = pred.shape[0]
    p3 = pred.reshape(B, H, W)
    t3 = target.reshape(B, H, W)

    tb = t3 != 0
    k_t = _needed_iters(_bin_soft_erode(tb), K_SKEL - 1, _bin_soft_erode) + 1
    k_t = min(k_t, K_SKEL)
    d_p = _needed_iters(p3 <= 0.0, 19, _bin_erode3)
    d_t = _needed_iters(~tb, 19, _bin_erode3)

    key = (k_t, d_p, d_t)
    if key not in _cache:
        _cache[key] = build(*key)
    nc = _cache[key]

    in_maps = [
        {"pred": p3[4 * c:4 * c + 4], "target": t3[4 * c:4 * c + 4]}
        for c in range(NCORES)
    ]
    res = run_bass_kernel_spmd(nc, in_maps, core_ids=list(range(NCORES)))
    st = np.stack([r["out"] for r in res.results])  # [8, 2, 128, STC]
    s = st.sum(axis=(0, 1, 2), dtype=np.float64)    # summed stats

    N = float(pred.size)
    smooth, eps, hsm = 1.0, 1.0, 1e-6
    sum_sp = -(s[C_SP] + s[C_SP + 1])
    sum_pt = s[C_PT]
    sum_p = s[C_P] + s[C_P + 1]
    inter = s[C_PROBT]
    sum_t = s[C_T] + s[C_T + 1]
    loss_bce = (sum_sp - sum_pt) / N
    loss_dice = 1.0 - (2.0 * inter + smooth) / (sum_p + sum_t + smooth)
    fp = sum_p - inter
    fn = sum_t - inter
    tversky = (inter + smooth) / (inter + 0.3 * fp + 0.7 * fn + smooth)
    loss_ft = (1.0 - tversky) ** 1.33
    loss_boundary = loss_bce + 3.0 * (-s[C_BSP] - s[C_BPT]) / N
    tprec = ((sum_t - s[C_SPT]) + eps) / ((N - s[C_SPS]) + eps)
    tsens = ((sum_p - s[C_STP]) + eps) / ((N - s[C_STS]) + eps)
    loss_cldice = 1.0 - 2.0 * tprec * tsens / (tprec + tsens)
    dtp = s[C_DTP]
    dtt = s[C_DTT] - s[C_DTTM]
    n_pb = N - s[C_MASK]
    hd_fwd = (dtp + hsm) / (sum_t + hsm)
    hd_bwd = (dtt + hsm) / (n_pb + hsm)
    loss_hd = 0.5 * (hd_fwd + hd_bwd)
    total = (0.2 * loss_bce + 0.2 * loss_dice + 0.2 * loss_cldice
             + 0.1 * loss_hd + 0.1 * loss_boundary + 0.2 * loss_ft)
    return np.float32(total)


# revision 16
# speedup vs baseline: 1.1058x; 1.1046x over previous
"""Trainium2 Bass kernel for nn_ComprehensiveLoss (BCE+Dice+FocalTversky+
Boundary+clDice+Hausdorff) on [32,1,512,512] inputs.

Strategy: pure data parallel over batch — 4 images per core as two
column-interleaved image pairs. All morphology (soft-skeleton, erosion
distance transforms, boundary) runs fused in SBUF in bf16. Cross-partition
halo rows are produced by TensorE shift-matmuls into PSUM and copied back by
the Scalar engine, so the Vector engine never waits on DMA. The two pairs'
instruction streams are interleaved so one pair's DVE work hides the other
pair's halo latency. Each core emits per-partition partial sums; the final
scalar ratios are combined on the host.

Layout: each image pair is column-interleaved (position 2c+img) so every
1-column stencil shift stays 4-byte aligned (keeps DVE 2x mode). Partition p
holds rows 4p..4p+3 of both images plus 2 halo rows (tile rows 0 and 5).
"""
import numpy as np
import concourse.bacc as bacc
import concourse.mybir as mybir
from concourse.tile import TileContext
from concourse.bass_utils import run_bass_kernel_spmd

F32 = mybir.dt.float32
BF16 = mybir.dt.bfloat16
I32 = mybir.dt.int32
OP = mybir.AluOpType
AF = mybir.ActivationFunctionType
AX = mybir.AxisListType

P = 128
NCORES = 8
H = W = 512
C2 = 2 * W           # interleaved row width
RPP = 4              # owned rows per partition
K_SKEL = 10          # reference soft_skeleton iters

# stats column map (per pair); ACT-accumulated stats get one column/image
C_SP = 0      # +i: sum softplus(pred)
C_P = 2       # +i: sum sigmoid(pred)
C_T = 4       # +i: sum t
C_PT = 6      # sum pred*t
C_PROBT = 7   # sum prob*t
C_MASK = 8    # sum (pred<=0)
C_BSP = 9     # sum boundary*softplus
C_BPT = 10    # sum boundary*pred*t
C_SPT = 11    # sum u_pred*t
C_SPS = 12    # sum u_pred
C_STP = 13    # sum u_true*prob
C_STS = 14    # sum u_true
C_DTP = 15    # sum dist_p*t
C_DTT = 16    # sum dist_t
C_DTTM = 17   # sum dist_t*mask_p
STC = 18


def _img(view, i):
    """image-i sub-view of an interleaved [...,1024] view"""
    return view.rearrange("p r (c two) -> p r c two", two=2)[:, :, :, i]


def _epair(v, a, b):
    """[P,4,1024] view -> positions {a,a+1,b,b+1} as [P,4,2,2] (b>a, even)"""
    g = v.rearrange("p r (g c) -> p r g c", c=2)
    return g[:, :, a // 2:b // 2 + 1:(b - a) // 2, :]


class _Pair:
    """Per-pair tiles + op emitters. All emitters are fine-grained so the
    build loop can interleave the two pairs' instruction streams."""

    def __init__(self, nc, pool, psum_pool, shm, pair):
        self.nc = nc
        self.shm = shm  # [128, 4, 128] bf16 shift matrices
        s = f"_{pair}"
        # 6-row tiles (owned rows 1..4, halo rows 0 and 5)
        self.T = pool.tile([P, 6, C2], BF16, name="T" + s, tag="T" + s)
        self.PR = pool.tile([P, 6, C2], BF16, name="PR" + s, tag="PR" + s)
        self.MK = pool.tile([P, 6, C2], BF16, name="MK" + s, tag="MK" + s)
        # E-slots double as phase-1 planar staging (PRD / TB) via shared tags
        self.PRD = pool.tile([P, 2, RPP, W], BF16, name="PRD" + s, tag="E1" + s)
        self.TB = pool.tile([P, 2, RPP, W], BF16, name="TB" + s, tag="E2" + s)
        # 4-row scratch
        self.A = pool.tile([P, RPP, C2], BF16, name="A" + s, tag="A" + s)
        self.B = pool.tile([P, RPP, C2], BF16, name="B" + s, tag="B" + s)
        self.C = pool.tile([P, RPP, C2], BF16, name="C" + s, tag="C" + s)
        # LU: logits in phase1/boundary, then u-product ping-pong slot 0
        self.LU = pool.tile([P, RPP, C2], BF16, name="LU" + s, tag="LU" + s)
        # SP: softplus image through boundary, then u ping-pong slot 1
        self.SP = pool.tile([P, RPP, C2], BF16, name="SP" + s, tag="SP" + s)
        self.ST = pool.tile([P, STC], F32, name="ST" + s, tag="ST" + s)
        self.psum = psum_pool
        self.pool = pool
        self.s = s
        self.E1 = None
        self.E2 = None

    def make_e_tiles(self):
        # allocated after PRD/TB are dead; same memory via shared tags
        self.E1 = self.pool.tile([P, 6, C2], BF16, name="E1t" + self.s,
                                 tag="E1" + self.s)
        self.E2 = self.pool.tile([P, 6, C2], BF16, name="E2t" + self.s,
                                 tag="E2" + self.s)

    # ---- halo fill via TensorE + ScalarE (replaces DMA refresh) ----
    def halo(self, X):
        """fill halo rows of 6-row tile X: row0 <- below-neighbor row4
        (clamp at p=0 to own row1), row5 <- above-neighbor row1 (clamp at
        p=127 to own row4)."""
        nc, shm = self.nc, self.shm
        pp = self.psum.tile([P, 2, W], F32, name="pp" + self.s,
                            tag="pp" + self.s)
        for cc in (0, W):
            top = pp[:, 0, :] if cc == 0 else pp[:, 1, :]
            nc.tensor.matmul(top, shm[:, 0, :], X[:, 4, cc:cc + W],
                             start=True, stop=False)
            nc.tensor.matmul(top, shm[:, 2, :], X[:, 1, cc:cc + W],
                             start=False, stop=True)
        pb = self.psum.tile([P, 2, W], F32, name="pb" + self.s,
                            tag="pb" + self.s)
        for cc in (0, W):
            bot = pb[:, 0, :] if cc == 0 else pb[:, 1, :]
            nc.tensor.matmul(bot, shm[:, 1, :], X[:, 1, cc:cc + W],
                             start=True, stop=False)
            nc.tensor.matmul(bot, shm[:, 3, :], X[:, 4, cc:cc + W],
                             start=False, stop=True)
        nc.scalar.activation(out=X[:, 0, :], in_=pp.rearrange("p a b -> p (a b)"),
                             func=AF.Copy)
        nc.scalar.activation(out=X[:, 5, :], in_=pb.rearrange("p a b -> p (a b)"),
                             func=AF.Copy)

    # ---- stencil helpers (scratch discipline: A, B internal) ----
    def vpool(self, X, op, out):
        """vertical 3-tap of 6-row X -> out [P,4,1024] (uses A)"""
        nc = self.nc
        nc.vector.tensor_tensor(out=self.A[:], in0=X[:, 0:4, :],
                                in1=X[:, 2:6, :], op=op)
        nc.vector.tensor_tensor(out=out[:], in0=self.A[:],
                                in1=X[:, 1:5, :], op=op)

    def hpool(self, IN, op, out, scratch):
        """horizontal 3-tap IN [P,4,1024] -> out (clamped edges).
        scratch must differ from IN; out may alias scratch."""
        nc = self.nc
        nc.vector.tensor_tensor(out=scratch[:, :, 2:1022], in0=IN[:, :, 0:1020],
                                in1=IN[:, :, 4:1024], op=op)
        nc.vector.tensor_tensor(out=out[:, :, 2:1022], in0=scratch[:, :, 2:1022],
                                in1=IN[:, :, 2:1022], op=op)
        nc.vector.tensor_tensor(
            out=_epair(out, 0, 1022), in0=_epair(IN, 0, 1020),
            in1=_epair(IN, 2, 1022), op=op)

    def erode3(self, X, OUT):
        """3x3 min of 6-row X -> OUT [P,4,1024] (uses A, B)"""
        self.vpool(X, OP.min, self.B)
        self.hpool(self.B, OP.min, OUT, self.A)

    def soft_erode5(self, X, DST):
        """plus-shape 5-point min, 6-row X -> DST 6-row owned rows"""
        nc, A, B, C = self.nc, self.A, self.B, self.C
        Xo, Do = X[:, 1:5, :], DST[:, 1:5, :]
        nc.vector.tensor_tensor(out=A[:], in0=X[:, 0:4, :], in1=X[:, 2:6, :],
                                op=OP.min)   # m1 = min(up,down)
        nc.vector.tensor_tensor(out=B[:, :, 2:1022], in0=Xo[:, :, 0:1020],
                                in1=Xo[:, :, 4:1024], op=OP.min)  # m2
        nc.vector.tensor_tensor(out=C[:, :, 2:1022], in0=A[:, :, 2:1022],
                                in1=B[:, :, 2:1022], op=OP.min)
        nc.vector.tensor_tensor(out=Do[:, :, 2:1022], in0=C[:, :, 2:1022],
                                in1=Xo[:, :, 2:1022], op=OP.min)
        # edges: se[c0] = min(m1[c0], x[c0], x[c1]); both sides in one op
        nc.vector.tensor_tensor(out=_epair(C, 0, 1022), in0=_epair(A, 0, 1022),
                                in1=_epair(Xo, 2, 1020), op=OP.min)
        nc.vector.tensor_tensor(out=_epair(Do, 0, 1022),
                                in0=_epair(C, 0, 1022),
                                in1=_epair(Xo, 0, 1022), op=OP.min)

    # ---- skeleton pieces (emitted per-iteration by the build loop) ----
    def skel_dilate_update(self, src, dst, k, u_cur, u_nxt):
        """open = dilate3(dst) -> C; u update vs src owned rows.
        1 - relu(src - open) = min(open - src, 0) + 1, so the factor costs
        one 2x TT + one 4x TS instead of 1x STTs."""
        nc, A, B, C = self.nc, self.A, self.B, self.C
        self.vpool(dst, OP.max, B)
        self.hpool(B, OP.max, C, A)        # open -> C
        nc.vector.tensor_tensor(out=B[:], in0=C[:], in1=src[:, 1:5, :],
                                op=OP.subtract)     # nd = open - src
        tgt = u_nxt if k == 0 else A
        nc.vector.tensor_scalar(out=tgt[:], in0=B[:], scalar1=0.0,
                                scalar2=1.0, op0=OP.min, op1=OP.add)
        if k > 0:
            nc.vector.tensor_tensor(out=u_nxt[:], in0=A[:], in1=u_cur[:],
                                    op=OP.mult)


def build(k_t, d_p, d_t):
    nc = bacc.Bacc("TRN2", target_bir_lowering=False, debug=False,
                   num_devices=NCORES)
    pred_d = nc.dram_tensor("pred", [2 * 2, H, W], F32, kind="ExternalInput")
    targ_d = nc.dram_tensor("target", [2 * 2, H, W], I32, kind="ExternalInput")
    out_d = nc.dram_tensor("out", [2, P, STC], F32, kind="ExternalOutput")

    with TileContext(nc) as tc, \
            tc.tile_pool(name="main", bufs=1) as pool, \
            tc.tile_pool(name="psum", bufs=1, space="PSUM") as psum_pool:
        # ---- shift matrices for halo matmuls ----
        idx = pool.tile([P, 2, P], I32, name="idx", tag="idx")
        shm = pool.tile([P, 4, P], BF16, name="shm", tag="shm")
        nc.gpsimd.iota(idx[:, 0, :], pattern=[[1, P]], base=0,
                       channel_multiplier=-1)   # f - p
        nc.gpsimd.iota(idx[:, 1, :], pattern=[[1, P]], base=0,
                       channel_multiplier=1)    # f + p
        for j, (sl, val) in enumerate(((0, 1), (0, -1), (1, 0), (1, 254))):
            nc.vector.tensor_scalar(out=shm[:, j, :], in0=idx[:, sl, :],
                                    scalar1=val, scalar2=None, op0=OP.is_equal)

        bld = [_Pair(nc, pool, psum_pool, shm, p) for p in range(2)]

        # ---- input DMA (planar, with dtype cast) ----
        for p, b in enumerate(bld):
            pv = pred_d[2 * p:2 * p + 2].rearrange("i (p r) c -> p i r c", p=P)
            nc.gpsimd.dma_start(out=b.PRD[:], in_=pv)       # f32 -> bf16
            tv = targ_d[2 * p:2 * p + 2].rearrange("i (p r) c -> p i r c", p=P)
            nc.gpsimd.dma_start(out=b.TB[:], in_=tv)        # i32 -> bf16

        # ---- phase 1: pointwise stats + interleaved images (ACT-heavy) ----
        for b in bld:                     # copies first (Copy table)
            To = b.T[:, 1:5, :]
            for i in range(2):
                nc.scalar.activation(out=_img(To, i), in_=b.TB[:, i],
                                     func=AF.Copy,
                                     accum_out=b.ST[:, C_T + i:C_T + i + 1])
            for i in range(2):
                nc.scalar.activation(out=_img(b.LU[:], i), in_=b.PRD[:, i],
                                     func=AF.Copy)
        for b in bld:
            PRo = b.PR[:, 1:5, :]
            for i in range(2):
                nc.scalar.activation(out=_img(PRo, i), in_=b.PRD[:, i],
                                     func=AF.Sigmoid,
                                     accum_out=b.ST[:, C_P + i:C_P + i + 1])
            # softplus(x) = -ln(sigmoid(-x)); SP holds the NEGATED image
            # (host flips signs of C_SP / C_BSP)
            for i in range(2):
                nc.scalar.activation(out=_img(b.C[:], i), in_=b.PRD[:, i],
                                     func=AF.Sigmoid, scale=-1.0)
        for b in bld:
            for i in range(2):
                nc.scalar.activation(out=_img(b.SP[:], i), in_=_img(b.C[:], i),
                                     func=AF.Ln,
                                     accum_out=b.ST[:, C_SP + i:C_SP + i + 1])
        for b in bld:
            To = b.T[:, 1:5, :]
            # sum(pred*t), sum(prob*t), mask + sum(mask); scratch A/B
            nc.vector.tensor_tensor(out=b.A[:], in0=b.LU[:], in1=To,
                                    op=OP.mult)
            nc.scalar.activation(out=b.B[:], in_=b.A[:], func=AF.Copy,
                                 accum_out=b.ST[:, C_PT:C_PT + 1])
            nc.vector.tensor_tensor(out=b.C[:], in0=b.PR[:, 1:5, :], in1=To,
                                    op=OP.mult)
            nc.scalar.activation(out=b.B[:], in_=b.C[:], func=AF.Copy,
                                 accum_out=b.ST[:, C_PROBT:C_PROBT + 1])
            nc.vector.tensor_scalar(out=b.MK[:, 1:5, :], in0=b.LU[:],
                                    scalar1=0.0, scalar2=0.0, op0=OP.is_le,
                                    op1=OP.add,
                                    accum_out=b.ST[:, C_MASK:C_MASK + 1])
        for b in bld:
            b.halo(b.T)
            b.halo(b.PR)
            b.halo(b.MK)

        # ---- boundary loss sums ----
        for b in bld:
            nc_, To = b.nc, b.T[:, 1:5, :]
            b.make_e_tiles()  # PRD/TB dead from here (tag-shared memory)
            b.vpool(b.T, OP.max, b.B)
            b.hpool(b.B, OP.max, b.C, b.A)        # dilate -> C
            b.vpool(b.T, OP.min, b.B)
            b.hpool(b.B, OP.min, b.E1[:, 1:5, :], b.A)  # erode -> E1 owned
            nc_.vector.tensor_tensor(out=b.B[:], in0=b.C[:],
                                     in1=b.E1[:, 1:5, :],
                                     op=OP.subtract)          # b -> B
            nc_.vector.tensor_tensor(out=b.C[:], in0=b.B[:], in1=To,
                                     op=OP.mult)              # b*t -> C
            nc_.vector.tensor_tensor(out=b.A[:], in0=b.B[:], in1=b.SP[:],
                                     op=OP.mult)
            nc_.scalar.activation(out=b.E1[:, 1:5, :], in_=b.A[:],
                                  func=AF.Copy,
                                  accum_out=b.ST[:, C_BSP:C_BSP + 1])
            nc_.vector.tensor_tensor(out=b.B[:], in0=b.C[:], in1=b.LU[:],
                                     op=OP.mult)
            nc_.scalar.activation(out=b.E1[:, 1:5, :], in_=b.B[:],
                                  func=AF.Copy,
                                  accum_out=b.ST[:, C_BPT:C_BPT + 1])

        # ---- skeletons (pair-interleaved per iteration) ----
        def run_skeleton(src0_of, iters, w_of, col_prod, col_sum):
            srcs = {b: src0_of(b) for b in bld}
            ucur = {b: None for b in bld}
            for k in range(iters + 1):
                dsts = {b: (b.E1 if k % 2 == 0 else b.E2) for b in bld}
                for b in bld:
                    b.soft_erode5(srcs[b], dsts[b])
                for b in bld:
                    b.halo(dsts[b])
                for b in bld:
                    unxt = b.LU if k % 2 == 0 else b.SP
                    b.skel_dilate_update(srcs[b], dsts[b], k, ucur[b], unxt)
                    ucur[b] = unxt
                    srcs[b] = dsts[b]
            for b in bld:
                u = ucur[b]
                nc.vector.tensor_tensor(out=b.A[:], in0=u[:], in1=w_of(b),
                                        op=OP.mult)
                nc.scalar.activation(out=b.B[:], in_=b.A[:], func=AF.Copy,
                                     accum_out=b.ST[:, col_prod:col_prod + 1])
                nc.scalar.activation(out=b.C[:], in_=u[:], func=AF.Copy,
                                     accum_out=b.ST[:, col_sum:col_sum + 1])

        run_skeleton(lambda b: b.PR, K_SKEL, lambda b: b.T[:, 1:5, :],
                     C_SPT, C_SPS)
        run_skeleton(lambda b: b.T, k_t, lambda b: b.PR[:, 1:5, :],
                     C_STP, C_STS)

        # ---- distance transforms (pair-interleaved per iteration) ----
        def run_dt(m0_of, iters, finalize):
            srcs = {b: m0_of(b) for b in bld}
            accs = {b: None for b in bld}
            for d in range(1, iters + 1):
                dsts = {b: (b.E1 if d % 2 == 1 else b.E2) for b in bld}
                for b in bld:
                    b.erode3(srcs[b], dsts[b][:, 1:5, :])
                if d < iters:
                    for b in bld:
                        b.halo(dsts[b])
                for b in bld:
                    prev = (m0_of(b)[:, 1:5, :] if accs[b] is None
                            else accs[b][:])
                    acc_n = b.C if d % 2 == 1 else b.LU
                    nc.vector.tensor_add(out=acc_n[:], in0=prev,
                                         in1=dsts[b][:, 1:5, :])
                    accs[b] = acc_n
                    srcs[b] = dsts[b]
            for b in bld:
                final = (m0_of(b)[:, 1:5, :] if accs[b] is None
                         else accs[b][:])
                finalize(b, final)

        # DT of pred mask (MK), weighted by t
        def fin_p(b, final):
            nc.vector.tensor_tensor(out=b.B[:], in0=final,
                                    in1=b.T[:, 1:5, :], op=OP.mult)
            nc.scalar.activation(out=b.A[:], in_=b.B[:], func=AF.Copy,
                                 accum_out=b.ST[:, C_DTP:C_DTP + 1])
        run_dt(lambda b: b.MK, d_p, fin_p)

        # DT of (1 - t): overwrite T with its complement; weight = 1 - mask_p
        for b in bld:
            nc.vector.tensor_scalar(out=b.T[:, 1:5, :], in0=b.T[:, 1:5, :],
                                    scalar1=-1.0, scalar2=1.0, op0=OP.mult,
                                    op1=OP.add)
        for b in bld:
            b.halo(b.T)

        def fin_t(b, final):
            nc.scalar.activation(out=b.A[:], in_=final, func=AF.Copy,
                                 accum_out=b.ST[:, C_DTT:C_DTT + 1])
            nc.vector.tensor_tensor(out=b.B[:], in0=final,
                                    in1=b.MK[:, 1:5, :], op=OP.mult)
            nc.scalar.activation(out=b.C[:], in_=b.B[:], func=AF.Copy,
                                 accum_out=b.ST[:, C_DTTM:C_DTTM + 1])
        run_dt(lambda b: b.T, d_t, fin_t)

        for p, b in enumerate(bld):
            nc.sync.dma_start(out=out_d[p], in_=b.ST[:])
    nc.compile()
    return nc


# ---------------- host side ----------------
_cache = {}


def _bin_soft_erode(e):
    v = e & np.roll(e, 1, 1) & np.roll(e, -1, 1)
    v[:, 0] = e[:, 0] & e[:, 1]
    v[:, -1] = e[:, -1] & e[:, -2]
    h = e & np.roll(e, 1, 2) & np.roll(e, -1, 2)
    h[:, :, 0] = e[:, :, 0] & e[:, :, 1]
    h[:, :, -1] = e[:, :, -1] & e[:, :, -2]
    return v | h


def _bin_erode3(e):
    v = e & np.roll(e, 1, 1) & np.roll(e, -1, 1)
    v[:, 0] = e[:, 0] & e[:, 1]
    v[:, -1] = e[:, -1] & e[:, -2]
    h = v & np.roll(v, 1, 2) & np.roll(v, -1, 2)
    h[:, :, 0] = v[:, :, 0] & v[:, :, 1]
    h[:, :, -1] = v[:, :, -1] & v[:, :, -2]
    return h


def _needed_iters(mask, limit, erode_fn):
    """number of erosions until empty (capped)"""
    e, n = mask, 0
    while n < limit:
        e = erode_fn(e)
        if not e.any():
            break
        n += 1
    return n


def kernel(pred, target):
    pred = np.ascontiguousarray(np.asarray(pred), dtype=np.float32)
    target = np.ascontiguousarray(np.asarray(target), dtype=np.int32)
    B = pred.shape[0]
    p3 = pred.reshape(B, H, W)
    t3 = target.reshape(B, H, W)

    tb = t3 != 0
    k_t = _needed_iters(_bin_soft_erode(tb), K_SKEL - 1, _bin_soft_erode) + 1
    k_t = min(k_t, K_SKEL)
    d_p = _needed_iters(p3 <= 0.0, 19, _bin_erode3)
    d_t = _needed_iters(~tb, 19, _bin_erode3)

    key = (k_t, d_p, d_t)
    if key not in _cache:
        _cache[key] = build(*key)
    nc = _cache[key]

    in_maps = [
        {"pred": p3[4 * c:4 * c + 4], "target": t3[4 * c:4 * c + 4]}
        for c in range(NCORES)
    ]
    res = run_bass_kernel_spmd(nc, in_maps, core_ids=list(range(NCORES)))
    st = np.stack([r["out"] for r in res.results])  # [8, 2, 128, STC]
    s = st.sum(axis=(0, 1, 2), dtype=np.float64)    # summed stats

    N = float(pred.size)
    smooth, eps, hsm = 1.0, 1.0, 1e-6
    sum_sp = -(s[C_SP] + s[C_SP + 1])
    sum_pt = s[C_PT]
    sum_p = s[C_P] + s[C_P + 1]
    inter = s[C_PROBT]
    sum_t = s[C_T] + s[C_T + 1]
    loss_bce = (sum_sp - sum_pt) / N
    loss_dice = 1.0 - (2.0 * inter + smooth) / (sum_p + sum_t + smooth)
    fp = sum_p - inter
    fn = sum_t - inter
    tversky = (inter + smooth) / (inter + 0.3 * fp + 0.7 * fn + smooth)
    loss_ft = (1.0 - tversky) ** 1.33
    loss_boundary = loss_bce + 3.0 * (-s[C_BSP] - s[C_BPT]) / N
    tprec = ((sum_t - s[C_SPT]) + eps) / ((N - s[C_SPS]) + eps)
    tsens = ((sum_p - s[C_STP]) + eps) / ((N - s[C_STS]) + eps)
    loss_cldice = 1.0 - 2.0 * tprec * tsens / (tprec + tsens)
    dtp = s[C_DTP]
    dtt = s[C_DTT] - s[C_DTTM]
    n_pb = N - s[C_MASK]
    hd_fwd = (dtp + hsm) / (sum_t + hsm)
    hd_bwd = (dtt + hsm) / (n_pb + hsm)
    loss_hd = 0.5 * (hd_fwd + hd_bwd)
    total = (0.2 * loss_bce + 0.2 * loss_dice + 0.2 * loss_cldice
             + 0.1 * loss_hd + 0.1 * loss_boundary + 0.2 * loss_ft)
    return np.float32(total)


# revision 20
# speedup vs baseline: 1.1648x; 1.0534x over previous
"""Trainium2 Bass kernel for nn_ComprehensiveLoss (BCE+Dice+FocalTversky+
Boundary+clDice+Hausdorff) on [32,1,512,512] inputs.

Strategy: pure data parallel over batch — 4 images per core as two
column-interleaved image pairs. All morphology (soft-skeleton, erosion
distance transforms, boundary) runs fused in SBUF in bf16. Cross-partition
halo rows are produced by TensorE shift-matmuls into PSUM and copied back by
the Scalar engine, so the Vector engine never waits on DMA. The two pairs'
instruction streams are interleaved so one pair's DVE work hides the other
pair's halo latency. Each core emits per-partition partial sums; the final
scalar ratios are combined on the host.

Layout: each image pair is column-interleaved (position 2c+img) so every
1-column stencil shift stays 4-byte aligned (keeps DVE 2x mode). Partition p
holds rows 4p..4p+3 of both images plus 2 halo rows (tile rows 0 and 5).
"""
import numpy as np
import concourse.bacc as bacc
import concourse.mybir as mybir
from concourse.tile import TileContext
from concourse.bass_utils import run_bass_kernel_spmd

F32 = mybir.dt.float32
BF16 = mybir.dt.bfloat16
I32 = mybir.dt.int32
OP = mybir.AluOpType
AF = mybir.ActivationFunctionType
AX = mybir.AxisListType

P = 128
NCORES = 8
H = W = 512
C2 = 2 * W           # interleaved row width
RPP = 4              # owned rows per partition
K_SKEL = 10          # reference soft_skeleton iters

# stats column map (per pair); ACT-accumulated stats get one column/image
C_SP = 0      # +i: sum softplus(pred)
C_P = 2       # +i: sum sigmoid(pred)
C_T = 4       # +i: sum t
C_PT = 6      # sum pred*t
C_PROBT = 7   # sum prob*t
C_MASK = 8    # sum (pred<=0)
C_BSP = 9     # sum boundary*softplus
C_BPT = 10    # sum boundary*pred*t
C_SPT = 11    # sum u_pred*t
C_SPS = 12    # sum u_pred
C_STP = 13    # sum u_true*prob
C_STS = 14    # sum u_true
C_DTP = 15    # sum dist_p*t
C_DTT = 16    # sum dist_t
C_DTTM = 17   # sum dist_t*mask_p
STC = 18


def _img(view, i):
    """image-i sub-view of an interleaved [...,1024] view"""
    return view.rearrange("p r (c two) -> p r c two", two=2)[:, :, :, i]


def _epair(v, a, b):
    """[P,4,1024] view -> positions {a,a+1,b,b+1} as [P,4,2,2] (b>a, even)"""
    g = v.rearrange("p r (g c) -> p r g c", c=2)
    return g[:, :, a // 2:b // 2 + 1:(b - a) // 2, :]


class _Pair:
    """Per-pair tiles + op emitters. All emitters are fine-grained so the
    build loop can interleave the two pairs' instruction streams."""

    def __init__(self, nc, pool, psum_pool, shm, pair):
        self.nc = nc
        self.shm = shm  # [128, 4, 128] bf16 shift matrices
        s = f"_{pair}"
        # 6-row tiles (owned rows 1..4, halo rows 0 and 5)
        self.T = pool.tile([P, 6, C2], BF16, name="T" + s, tag="T" + s)
        self.PR = pool.tile([P, 6, C2], BF16, name="PR" + s, tag="PR" + s)
        self.MK = pool.tile([P, 6, C2], BF16, name="MK" + s, tag="MK" + s)
        # E-slots double as phase-1 planar staging (PRD / TB) via shared tags
        self.PRD = pool.tile([P, 2, RPP, W], BF16, name="PRD" + s, tag="E1" + s)
        self.TB = pool.tile([P, 2, RPP, W], BF16, name="TB" + s, tag="E2" + s)
        # 4-row scratch
        self.A = pool.tile([P, RPP, C2], BF16, name="A" + s, tag="A" + s)
        self.B = pool.tile([P, RPP, C2], BF16, name="B" + s, tag="B" + s)
        self.C = pool.tile([P, RPP, C2], BF16, name="C" + s, tag="C" + s)
        # LU: logits in phase1/boundary, then u-product ping-pong slot 0
        self.LU = pool.tile([P, RPP, C2], BF16, name="LU" + s, tag="LU" + s)
        # SP: softplus image through boundary, then u ping-pong slot 1
        self.SP = pool.tile([P, RPP, C2], BF16, name="SP" + s, tag="SP" + s)
        self.ST = pool.tile([P, STC], F32, name="ST" + s, tag="ST" + s)
        self.psum = psum_pool
        self.pool = pool
        self.s = s
        self.E1 = None
        self.E2 = None

    def make_e_tiles(self):
        # allocated after PRD/TB are dead; same memory via shared tags
        self.E1 = self.pool.tile([P, 6, C2], BF16, name="E1t" + self.s,
                                 tag="E1" + self.s)
        self.E2 = self.pool.tile([P, 6, C2], BF16, name="E2t" + self.s,
                                 tag="E2" + self.s)

    # ---- halo fill via TensorE + ScalarE (replaces DMA refresh) ----
    def halo(self, X):
        """fill halo rows of 6-row tile X: row0 <- below-neighbor row4
        (clamp at p=0 to own row1), row5 <- above-neighbor row1 (clamp at
        p=127 to own row4). Shift matmuls cover partitions 1..127 / 0..126;
        the two clamp rows go through the idle GpSimd engine in parallel."""
        nc, shm = self.nc, self.shm
        pp = self.psum.tile([P, 2, W], F32, name="pp" + self.s,
                            tag="pp" + self.s)
        for cc in (0, W):
            top = pp[:, cc // W, :]
            nc.tensor.matmul(top, shm[:, 0, :], X[:, 4, cc:cc + W],
                             start=True, stop=False)
            nc.tensor.matmul(top, shm[:, 2, :], X[:, 1, cc:cc + W],
                             start=False, stop=True)
        pb = self.psum.tile([P, 2, W], F32, name="pb" + self.s,
                            tag="pb" + self.s)
        for cc in (0, W):
            bot = pb[:, cc // W, :]
            nc.tensor.matmul(bot, shm[:, 1, :], X[:, 1, cc:cc + W],
                             start=True, stop=False)
            nc.tensor.matmul(bot, shm[:, 3, :], X[:, 4, cc:cc + W],
                             start=False, stop=True)
        nc.scalar.activation(out=X[:, 0, :],
                             in_=pp.rearrange("p a b -> p (a b)"),
                             func=AF.Copy)
        nc.scalar.activation(out=X[:, 5, :],
                             in_=pb.rearrange("p a b -> p (a b)"),
                             func=AF.Copy)

    # ---- stencil helpers (scratch discipline: A, B internal) ----
    def vpool(self, X, op, out):
        """vertical 3-tap of 6-row X -> out [P,4,1024] (uses A)"""
        nc = self.nc
        nc.vector.tensor_tensor(out=self.A[:], in0=X[:, 0:4, :],
                                in1=X[:, 2:6, :], op=op)
        nc.vector.tensor_tensor(out=out[:], in0=self.A[:],
                                in1=X[:, 1:5, :], op=op)

    def hpool(self, IN, op, out, scratch):
        """horizontal 3-tap IN [P,4,1024] -> out (clamped edges).
        scratch must differ from IN; out may alias scratch."""
        nc = self.nc
        nc.vector.tensor_tensor(out=scratch[:, :, 2:1022], in0=IN[:, :, 0:1020],
                                in1=IN[:, :, 4:1024], op=op)
        nc.vector.tensor_tensor(out=out[:, :, 2:1022], in0=scratch[:, :, 2:1022],
                                in1=IN[:, :, 2:1022], op=op)
        nc.vector.tensor_tensor(
            out=_epair(out, 0, 1022), in0=_epair(IN, 0, 1020),
            in1=_epair(IN, 2, 1022), op=op)

    def erode3(self, X, OUT):
        """3x3 min of 6-row X -> OUT [P,4,1024] (uses A, B)"""
        self.vpool(X, OP.min, self.B)
        self.hpool(self.B, OP.min, OUT, self.A)

    def soft_erode5(self, X, DST):
        """plus-shape 5-point min, 6-row X -> DST 6-row owned rows.
        DST rows 1 and 4 are written first so the halo matmuls (which read
        exactly those rows) can launch while rows 2-3 are still computing."""
        nc, A, B, C = self.nc, self.A, self.B, self.C
        Xo, Do = X[:, 1:5, :], DST[:, 1:5, :]
        nc.vector.tensor_tensor(out=A[:], in0=X[:, 0:4, :], in1=X[:, 2:6, :],
                                op=OP.min)   # m1 = min(up,down)
        nc.vector.tensor_tensor(out=B[:, :, 2:1022], in0=Xo[:, :, 0:1020],
                                in1=Xo[:, :, 4:1024], op=OP.min)  # m2
        nc.vector.tensor_tensor(out=C[:, :, 2:1022], in0=A[:, :, 2:1022],
                                in1=B[:, :, 2:1022], op=OP.min)
        # edges: se[c0] = min(m1[c0], x[c0], x[c1]); both sides in one op
        nc.vector.tensor_tensor(out=_epair(C, 0, 1022), in0=_epair(A, 0, 1022),
                                in1=_epair(Xo, 2, 1020), op=OP.min)
        for rs in (slice(0, 4, 3), slice(1, 3)):   # rows {1,4} first
            nc.vector.tensor_tensor(out=Do[:, rs, 2:1022],
                                    in0=C[:, rs, 2:1022],
                                    in1=Xo[:, rs, 2:1022], op=OP.min)
            nc.vector.tensor_tensor(out=_epair(Do[:, rs], 0, 1022),
                                    in0=_epair(C[:, rs], 0, 1022),
                                    in1=_epair(Xo[:, rs], 0, 1022), op=OP.min)

    # ---- skeleton pieces (emitted per-iteration by the build loop) ----
    def skel_dilate_update(self, src, dst, k, u_cur, u_nxt):
        """open = dilate3(dst) -> C; u update vs src owned rows.
        1 - relu(src - open) = min(open - src, 0) + 1, so the factor costs
        one 2x TT + one 4x TS instead of 1x STTs."""
        nc, A, B, C = self.nc, self.A, self.B, self.C
        self.vpool(dst, OP.max, B)
        self.hpool(B, OP.max, C, A)        # open -> C
        nc.vector.tensor_tensor(out=B[:], in0=C[:], in1=src[:, 1:5, :],
                                op=OP.subtract)     # nd = open - src
        tgt = u_nxt if k == 0 else A
        nc.vector.tensor_scalar(out=tgt[:], in0=B[:], scalar1=0.0,
                                scalar2=1.0, op0=OP.min, op1=OP.add)
        if k > 0:
            nc.vector.tensor_tensor(out=u_nxt[:], in0=A[:], in1=u_cur[:],
                                    op=OP.mult)


def build(k_t, d_p, d_t):
    nc = bacc.Bacc("TRN2", target_bir_lowering=False, debug=False,
                   num_devices=NCORES)
    pred_d = nc.dram_tensor("pred", [2 * 2, H, W], F32, kind="ExternalInput")
    targ_d = nc.dram_tensor("target", [2 * 2, H, W], I32, kind="ExternalInput")
    out_d = nc.dram_tensor("out", [2, P, STC], F32, kind="ExternalOutput")

    with TileContext(nc) as tc, \
            tc.tile_pool(name="main", bufs=1) as pool, \
            tc.tile_pool(name="psum", bufs=1, space="PSUM") as psum_pool:
        # ---- shift matrices for halo matmuls ----
        idx = pool.tile([P, 2, P], I32, name="idx", tag="idx")
        shm = pool.tile([P, 4, P], BF16, name="shm", tag="shm")
        nc.gpsimd.iota(idx[:, 0, :], pattern=[[1, P]], base=0,
                       channel_multiplier=-1)   # f - p
        nc.gpsimd.iota(idx[:, 1, :], pattern=[[1, P]], base=0,
                       channel_multiplier=1)    # f + p
        for j, (sl, val) in enumerate(((0, 1), (0, -1), (1, 0), (1, 254))):
            nc.vector.tensor_scalar(out=shm[:, j, :], in0=idx[:, sl, :],
                                    scalar1=val, scalar2=None, op0=OP.is_equal)

        bld = [_Pair(nc, pool, psum_pool, shm, p) for p in range(2)]

        # ---- input DMA (planar, with dtype cast) ----
        for p, b in enumerate(bld):
            pv = pred_d[2 * p:2 * p + 2].rearrange("i (p r) c -> p i r c", p=P)
            nc.gpsimd.dma_start(out=b.PRD[:], in_=pv)       # f32 -> bf16
            tv = targ_d[2 * p:2 * p + 2].rearrange("i (p r) c -> p i r c", p=P)
            nc.gpsimd.dma_start(out=b.TB[:], in_=tv)        # i32 -> bf16

        # ---- phase 1: pointwise stats + interleaved images (ACT-heavy) ----
        for b in bld:                     # copies first (Copy table)
            To = b.T[:, 1:5, :]
            for i in range(2):
                nc.scalar.activation(out=_img(To, i), in_=b.TB[:, i],
                                     func=AF.Copy,
                                     accum_out=b.ST[:, C_T + i:C_T + i + 1])
            for i in range(2):
                nc.scalar.activation(out=_img(b.LU[:], i), in_=b.PRD[:, i],
                                     func=AF.Copy)
        for b in bld:
            PRo = b.PR[:, 1:5, :]
            for i in range(2):
                nc.scalar.activation(out=_img(PRo, i), in_=b.PRD[:, i],
                                     func=AF.Sigmoid,
                                     accum_out=b.ST[:, C_P + i:C_P + i + 1])
            # softplus(x) = -ln(sigmoid(-x)); SP holds the NEGATED image
            # (host flips signs of C_SP / C_BSP)
            for i in range(2):
                nc.scalar.activation(out=_img(b.C[:], i), in_=b.PRD[:, i],
                                     func=AF.Sigmoid, scale=-1.0)
        for b in bld:
            for i in range(2):
                nc.scalar.activation(out=_img(b.SP[:], i), in_=_img(b.C[:], i),
                                     func=AF.Ln,
                                     accum_out=b.ST[:, C_SP + i:C_SP + i + 1])
        for b in bld:
            To = b.T[:, 1:5, :]
            # sum(pred*t), sum(prob*t), mask + sum(mask); scratch A/B
            nc.vector.tensor_tensor(out=b.A[:], in0=b.LU[:], in1=To,
                                    op=OP.mult)
            nc.scalar.activation(out=b.B[:], in_=b.A[:], func=AF.Copy,
                                 accum_out=b.ST[:, C_PT:C_PT + 1])
            nc.vector.tensor_tensor(out=b.C[:], in0=b.PR[:, 1:5, :], in1=To,
                                    op=OP.mult)
            nc.scalar.activation(out=b.B[:], in_=b.C[:], func=AF.Copy,
                                 accum_out=b.ST[:, C_PROBT:C_PROBT + 1])
            nc.vector.tensor_scalar(out=b.MK[:, 1:5, :], in0=b.LU[:],
                                    scalar1=0.0, scalar2=0.0, op0=OP.is_le,
                                    op1=OP.add,
                                    accum_out=b.ST[:, C_MASK:C_MASK + 1])
        for b in bld:
            b.halo(b.T)
            b.halo(b.PR)
            b.halo(b.MK)

        # ---- boundary loss sums ----
        for b in bld:
            nc_, To = b.nc, b.T[:, 1:5, :]
            b.make_e_tiles()  # PRD/TB dead from here (tag-shared memory)
            b.vpool(b.T, OP.max, b.B)
            b.hpool(b.B, OP.max, b.C, b.A)        # dilate -> C
            b.vpool(b.T, OP.min, b.B)
            b.hpool(b.B, OP.min, b.E1[:, 1:5, :], b.A)  # erode -> E1 owned
            nc_.vector.tensor_tensor(out=b.B[:], in0=b.C[:],
                                     in1=b.E1[:, 1:5, :],
                                     op=OP.subtract)          # b -> B
            nc_.vector.tensor_tensor(out=b.C[:], in0=b.B[:], in1=To,
                                     op=OP.mult)              # b*t -> C
            nc_.vector.tensor_tensor(out=b.A[:], in0=b.B[:], in1=b.SP[:],
                                     op=OP.mult)
            nc_.scalar.activation(out=b.E1[:, 1:5, :], in_=b.A[:],
                                  func=AF.Copy,
                                  accum_out=b.ST[:, C_BSP:C_BSP + 1])
            nc_.vector.tensor_tensor(out=b.B[:], in0=b.C[:], in1=b.LU[:],
                                     op=OP.mult)
            nc_.scalar.activation(out=b.E1[:, 1:5, :], in_=b.B[:],
                                  func=AF.Copy,
                                  accum_out=b.ST[:, C_BPT:C_BPT + 1])

        # ---- skeletons (pair-interleaved per iteration) ----
        def run_skeleton(src0_of, iters, w_of, col_prod, col_sum):
            srcs = {b: src0_of(b) for b in bld}
            ucur = {b: None for b in bld}
            for k in range(iters + 1):
                dsts = {b: (b.E1 if k % 2 == 0 else b.E2) for b in bld}
                for b in bld:
                    b.soft_erode5(srcs[b], dsts[b])
                for b in bld:
                    b.halo(dsts[b])
                for b in bld:
                    unxt = b.LU if k % 2 == 0 else b.SP
                    b.skel_dilate_update(srcs[b], dsts[b], k, ucur[b], unxt)
                    ucur[b] = unxt
                    srcs[b] = dsts[b]
            for b in bld:
                u = ucur[b]
                nc.vector.tensor_tensor(out=b.A[:], in0=u[:], in1=w_of(b),
                                        op=OP.mult)
                nc.scalar.activation(out=b.B[:], in_=b.A[:], func=AF.Copy,
                                     accum_out=b.ST[:, col_prod:col_prod + 1])
                nc.scalar.activation(out=b.C[:], in_=u[:], func=AF.Copy,
                                     accum_out=b.ST[:, col_sum:col_sum + 1])

        run_skeleton(lambda b: b.PR, K_SKEL, lambda b: b.T[:, 1:5, :],
                     C_SPT, C_SPS)
        run_skeleton(lambda b: b.T, k_t, lambda b: b.PR[:, 1:5, :],
                     C_STP, C_STS)

        # ---- distance transforms (pair-interleaved per iteration) ----
        def run_dt(m0_of, iters, finalize):
            srcs = {b: m0_of(b) for b in bld}
            accs = {b: None for b in bld}
            for d in range(1, iters + 1):
                dsts = {b: (b.E1 if d % 2 == 1 else b.E2) for b in bld}
                for b in bld:
                    b.erode3(srcs[b], dsts[b][:, 1:5, :])
                if d < iters:
                    for b in bld:
                        b.halo(dsts[b])
                for b in bld:
                    prev = (m0_of(b)[:, 1:5, :] if accs[b] is None
                            else accs[b][:])
                    acc_n = b.C if d % 2 == 1 else b.LU
                    nc.vector.tensor_add(out=acc_n[:], in0=prev,
                                         in1=dsts[b][:, 1:5, :])
                    accs[b] = acc_n
                    srcs[b] = dsts[b]
            for b in bld:
                final = (m0_of(b)[:, 1:5, :] if accs[b] is None
                         else accs[b][:])
                finalize(b, final)

        # DT of pred mask (MK), weighted by t
        def fin_p(b, final):
            nc.vector.tensor_tensor(out=b.B[:], in0=final,
                                    in1=b.T[:, 1:5, :], op=OP.mult)
            nc.scalar.activation(out=b.A[:], in_=b.B[:], func=AF.Copy,
                                 accum_out=b.ST[:, C_DTP:C_DTP + 1])
        run_dt(lambda b: b.MK, d_p, fin_p)

        # DT of (1 - t): overwrite T with its complement; weight = 1 - mask_p
        for b in bld:
            nc.vector.tensor_scalar(out=b.T[:, 1:5, :], in0=b.T[:, 1:5, :],
                                    scalar1=-1.0, scalar2=1.0, op0=OP.mult,
                                    op1=OP.add)
        for b in bld:
            b.halo(b.T)

        def fin_t(b, final):
            nc.scalar.activation(out=b.A[:], in_=final, func=AF.Copy,
                                 accum_out=b.ST[:, C_DTT:C_DTT + 1])
            nc.vector.tensor_tensor(out=b.B[:], in0=final,
                                    in1=b.MK[:, 1:5, :], op=OP.mult)
            nc.scalar.activation(out=b.C[:], in_=b.B[:], func=AF.Copy,
                                 accum_out=b.ST[:, C_DTTM:C_DTTM + 1])
        run_dt(lambda b: b.T, d_t, fin_t)

        for p, b in enumerate(bld):
            nc.sync.dma_start(out=out_d[p], in_=b.ST[:])
    nc.compile()
    return nc


# ---------------- host side ----------------
_cache = {}


def _bin_soft_erode(e):
    v = e & np.roll(e, 1, 1) & np.roll(e, -1, 1)
    v[:, 0] = e[:, 0] & e[:, 1]
    v[:, -1] = e[:, -1] & e[:, -2]
    h = e & np.roll(e, 1, 2) & np.roll(e, -1, 2)
    h[:, :, 0] = e[:, :, 0] & e[:, :, 1]
    h[:, :, -1] = e[:, :, -1] & e[:, :, -2]
    return v | h


def _bin_erode3(e):
    v = e & np.roll(e, 1, 1) & np.roll(e, -1, 1)
    v[:, 0] = e[:, 0] & e[:, 1]
    v[:, -1] = e[:, -1] & e[:, -2]
    h = v & np.roll(v, 1, 2) & np.roll(v, -1, 2)
    h[:, :, 0] = v[:, :, 0] & v[:, :, 1]
    h[:, :, -1] = v[:, :, -1] & v[:, :, -2]
    return h


def _needed_iters(mask, limit, erode_fn):
    """number of erosions until empty (capped)"""
    e, n = mask, 0
    while n < limit:
        e = erode_fn(e)
        if not e.any():
            break
        n += 1
    return n


def kernel(pred, target):
    pred = np.ascontiguousarray(np.asarray(pred), dtype=np.float32)
    target = np.ascontiguousarray(np.asarray(target), dtype=np.int32)
    B = pred.shape[0]
    p3 = pred.reshape(B, H, W)
    t3 = target.reshape(B, H, W)

    tb = t3 != 0
    k_t = _needed_iters(_bin_soft_erode(tb), K_SKEL - 1, _bin_soft_erode) + 1
    k_t = min(k_t, K_SKEL)
    d_p = _needed_iters(p3 <= 0.0, 19, _bin_erode3)
    d_t = _needed_iters(~tb, 19, _bin_erode3)

    key = (k_t, d_p, d_t)
    if key not in _cache:
        _cache[key] = build(*key)
    nc = _cache[key]

    in_maps = [
        {"pred": p3[4 * c:4 * c + 4], "target": t3[4 * c:4 * c + 4]}
        for c in range(NCORES)
    ]
    res = run_bass_kernel_spmd(nc, in_maps, core_ids=list(range(NCORES)))
    st = np.stack([r["out"] for r in res.results])  # [8, 2, 128, STC]
    s = st.sum(axis=(0, 1, 2), dtype=np.float64)    # summed stats

    N = float(pred.size)
    smooth, eps, hsm = 1.0, 1.0, 1e-6
    sum_sp = -(s[C_SP] + s[C_SP + 1])
    sum_pt = s[C_PT]
    sum_p = s[C_P] + s[C_P + 1]
    inter = s[C_PROBT]
    sum_t = s[C_T] + s[C_T + 1]
    loss_bce = (sum_sp - sum_pt) / N
    loss_dice = 1.0 - (2.0 * inter + smooth) / (sum_p + sum_t + smooth)
    fp = sum_p - inter
    fn = sum_t - inter
    tversky = (inter + smooth) / (inter + 0.3 * fp + 0.7 * fn + smooth)
    loss_ft = (1.0 - tversky) ** 1.33
    loss_boundary = loss_bce + 3.0 * (-s[C_BSP] - s[C_BPT]) / N
    tprec = ((sum_t - s[C_SPT]) + eps) / ((N - s[C_SPS]) + eps)
    tsens = ((sum_p - s[C_STP]) + eps) / ((N - s[C_STS]) + eps)
    loss_cldice = 1.0 - 2.0 * tprec * tsens / (tprec + tsens)
    dtp = s[C_DTP]
    dtt = s[C_DTT] - s[C_DTTM]
    n_pb = N - s[C_MASK]
    hd_fwd = (dtp + hsm) / (sum_t + hsm)
    hd_bwd = (dtt + hsm) / (n_pb + hsm)
    loss_hd = 0.5 * (hd_fwd + hd_bwd)
    total = (0.2 * loss_bce + 0.2 * loss_dice + 0.2 * loss_cldice
             + 0.1 * loss_hd + 0.1 * loss_boundary + 0.2 * loss_ft)
    return np.float32(total)


# revision 21
# speedup vs baseline: 1.1677x; 1.0025x over previous
"""Trainium2 Bass kernel for nn_ComprehensiveLoss (BCE+Dice+FocalTversky+
Boundary+clDice+Hausdorff) on [32,1,512,512] inputs.

Strategy: pure data parallel over batch — 4 images per core as two
column-interleaved image pairs. All morphology (soft-skeleton, erosion
distance transforms, boundary) runs fused in SBUF in bf16. Cross-partition
halo rows are produced by TensorE shift-matmuls into PSUM and copied back by
the Scalar engine, so the Vector engine never waits on DMA. The two pairs'
instruction streams are interleaved so one pair's DVE work hides the other
pair's halo latency. Each core emits per-partition partial sums; the final
scalar ratios are combined on the host.

Layout: each image pair is column-interleaved (position 2c+img) so every
1-column stencil shift stays 4-byte aligned (keeps DVE 2x mode). Partition p
holds rows 4p..4p+3 of both images plus 2 halo rows (tile rows 0 and 5).
"""
import numpy as np
import concourse.bacc as bacc
import concourse.mybir as mybir
from concourse.tile import TileContext
from concourse.bass_utils import run_bass_kernel_spmd

F32 = mybir.dt.float32
BF16 = mybir.dt.bfloat16
I32 = mybir.dt.int32
OP = mybir.AluOpType
AF = mybir.ActivationFunctionType
AX = mybir.AxisListType

P = 128
NCORES = 8
H = W = 512
C2 = 2 * W           # interleaved row width
RPP = 4              # owned rows per partition
K_SKEL = 10          # reference soft_skeleton iters

# stats column map (per pair); ACT-accumulated stats get one column/image
C_SP = 0      # +i: sum softplus(pred)
C_P = 2       # +i: sum sigmoid(pred)
C_T = 4       # +i: sum t
C_PT = 6      # sum pred*t
C_PROBT = 7   # sum prob*t
C_MASK = 8    # sum (pred<=0)
C_BSP = 9     # sum boundary*softplus
C_BPT = 10    # sum boundary*pred*t
C_SPT = 11    # sum u_pred*t
C_SPS = 12    # sum u_pred
C_STP = 13    # sum u_true*prob
C_STS = 14    # sum u_true
C_DTP = 15    # sum dist_p*t
C_DTT = 16    # sum dist_t
C_DTTM = 17   # sum dist_t*mask_p
STC = 18


def _img(view, i):
    """image-i sub-view of an interleaved [...,1024] view"""
    return view.rearrange("p r (c two) -> p r c two", two=2)[:, :, :, i]


def _epair(v, a, b):
    """[P,4,1024] view -> positions {a,a+1,b,b+1} as [P,4,2,2] (b>a, even)"""
    g = v.rearrange("p r (g c) -> p r g c", c=2)
    return g[:, :, a // 2:b // 2 + 1:(b - a) // 2, :]


class _Pair:
    """Per-pair tiles + op emitters. All emitters are fine-grained so the
    build loop can interleave the two pairs' instruction streams."""

    def __init__(self, nc, pool, psum_pool, shm, pair):
        self.nc = nc
        self.shm = shm  # [128, 4, 128] bf16 shift matrices
        s = f"_{pair}"
        # 6-row tiles (owned rows 1..4, halo rows 0 and 5)
        self.T = pool.tile([P, 6, C2], BF16, name="T" + s, tag="T" + s)
        self.PR = pool.tile([P, 6, C2], BF16, name="PR" + s, tag="PR" + s)
        self.MK = pool.tile([P, 6, C2], BF16, name="MK" + s, tag="MK" + s)
        # E-slots double as phase-1 planar staging (PRD / TB) via shared tags
        self.PRD = pool.tile([P, 2, RPP, W], BF16, name="PRD" + s, tag="E1" + s)
        self.TB = pool.tile([P, 2, RPP, W], BF16, name="TB" + s, tag="E2" + s)
        # 4-row scratch
        self.A = pool.tile([P, RPP, C2], BF16, name="A" + s, tag="A" + s)
        self.B = pool.tile([P, RPP, C2], BF16, name="B" + s, tag="B" + s)
        self.C = pool.tile([P, RPP, C2], BF16, name="C" + s, tag="C" + s)
        # LU: logits in phase1/boundary, then u-product ping-pong slot 0
        self.LU = pool.tile([P, RPP, C2], BF16, name="LU" + s, tag="LU" + s)
        # SP: softplus image through boundary, then u ping-pong slot 1
        self.SP = pool.tile([P, RPP, C2], BF16, name="SP" + s, tag="SP" + s)
        self.ST = pool.tile([P, STC], F32, name="ST" + s, tag="ST" + s)
        self.psum = psum_pool
        self.pool = pool
        self.s = s
        self.E1 = None
        self.E2 = None

    def make_e_tiles(self):
        # allocated after PRD/TB are dead; same memory via shared tags
        self.E1 = self.pool.tile([P, 6, C2], BF16, name="E1t" + self.s,
                                 tag="E1" + self.s)
        self.E2 = self.pool.tile([P, 6, C2], BF16, name="E2t" + self.s,
                                 tag="E2" + self.s)

    # ---- halo fill via TensorE + ScalarE (replaces DMA refresh) ----
    def halo(self, X):
        """fill halo rows of 6-row tile X: row0 <- below-neighbor row4
        (clamp at p=0 to own row1), row5 <- above-neighbor row1 (clamp at
        p=127 to own row4). Shift matmuls cover partitions 1..127 / 0..126;
        the two clamp rows go through the idle GpSimd engine in parallel."""
        nc, shm = self.nc, self.shm
        pp = self.psum.tile([P, 2, W], F32, name="pp" + self.s,
                            tag="pp" + self.s)
        for cc in (0, W):
            top = pp[:, cc // W, :]
            nc.tensor.matmul(top, shm[:, 0, :], X[:, 4, cc:cc + W],
                             start=True, stop=False)
            nc.tensor.matmul(top, shm[:, 2, :], X[:, 1, cc:cc + W],
                             start=False, stop=True)
        pb = self.psum.tile([P, 2, W], F32, name="pb" + self.s,
                            tag="pb" + self.s)
        for cc in (0, W):
            bot = pb[:, cc // W, :]
            nc.tensor.matmul(bot, shm[:, 1, :], X[:, 1, cc:cc + W],
                             start=True, stop=False)
            nc.tensor.matmul(bot, shm[:, 3, :], X[:, 4, cc:cc + W],
                             start=False, stop=True)
        nc.scalar.activation(out=X[:, 0, :],
                             in_=pp.rearrange("p a b -> p (a b)"),
                             func=AF.Copy)
        nc.scalar.activation(out=X[:, 5, :],
                             in_=pb.rearrange("p a b -> p (a b)"),
                             func=AF.Copy)

    # ---- stencil helpers (scratch discipline: A, B internal) ----
    def vpool(self, X, op, out):
        """vertical 3-tap of 6-row X -> out [P,4,1024] (uses A)"""
        nc = self.nc
        nc.vector.tensor_tensor(out=self.A[:], in0=X[:, 0:4, :],
                                in1=X[:, 2:6, :], op=op)
        nc.vector.tensor_tensor(out=out[:], in0=self.A[:],
                                in1=X[:, 1:5, :], op=op)

    def hpool(self, IN, op, out, scratch):
        """horizontal 3-tap IN [P,4,1024] -> out (clamped edges).
        scratch must differ from IN; out may alias scratch."""
        nc = self.nc
        nc.vector.tensor_tensor(out=scratch[:, :, 2:1022], in0=IN[:, :, 0:1020],
                                in1=IN[:, :, 4:1024], op=op)
        nc.vector.tensor_tensor(out=out[:, :, 2:1022], in0=scratch[:, :, 2:1022],
                                in1=IN[:, :, 2:1022], op=op)
        nc.vector.tensor_tensor(
            out=_epair(out, 0, 1022), in0=_epair(IN, 0, 1020),
            in1=_epair(IN, 2, 1022), op=op)

    def erode3(self, X, OUT):
        """3x3 min of 6-row X -> OUT [P,4,1024] (uses A, B)"""
        self.vpool(X, OP.min, self.B)
        self.hpool(self.B, OP.min, OUT, self.A)

    def soft_erode5(self, X, DST):
        """plus-shape 5-point min, 6-row X -> DST 6-row owned rows.
        DST rows 1 and 4 are written first so the halo matmuls (which read
        exactly those rows) can launch while rows 2-3 are still computing."""
        nc, A, B, C = self.nc, self.A, self.B, self.C
        Xo, Do = X[:, 1:5, :], DST[:, 1:5, :]
        nc.vector.tensor_tensor(out=A[:], in0=X[:, 0:4, :], in1=X[:, 2:6, :],
                                op=OP.min)   # m1 = min(up,down)
        nc.vector.tensor_tensor(out=B[:, :, 2:1022], in0=Xo[:, :, 0:1020],
                                in1=Xo[:, :, 4:1024], op=OP.min)  # m2
        nc.vector.tensor_tensor(out=C[:, :, 2:1022], in0=A[:, :, 2:1022],
                                in1=B[:, :, 2:1022], op=OP.min)
        # edges: se[c0] = min(m1[c0], x[c0], x[c1]); both sides in one op
        nc.vector.tensor_tensor(out=_epair(C, 0, 1022), in0=_epair(A, 0, 1022),
                                in1=_epair(Xo, 2, 1020), op=OP.min)
        for rs in (slice(0, 4, 3), slice(1, 3)):   # rows {1,4} first
            nc.vector.tensor_tensor(out=Do[:, rs, 2:1022],
                                    in0=C[:, rs, 2:1022],
                                    in1=Xo[:, rs, 2:1022], op=OP.min)
            nc.vector.tensor_tensor(out=_epair(Do[:, rs], 0, 1022),
                                    in0=_epair(C[:, rs], 0, 1022),
                                    in1=_epair(Xo[:, rs], 0, 1022), op=OP.min)

    # ---- skeleton pieces (emitted per-iteration by the build loop) ----
    def skel_dilate_update(self, src, dst, k, u_cur, u_nxt):
        """open = dilate3(dst) -> C; u update vs src owned rows.
        1 - relu(src - open) = min(open - src, 0) + 1, so the factor costs
        one 2x TT + one 4x TS instead of 1x STTs."""
        nc, A, B, C = self.nc, self.A, self.B, self.C
        self.vpool(dst, OP.max, B)
        self.hpool(B, OP.max, C, A)        # open -> C
        nc.vector.tensor_tensor(out=B[:], in0=C[:], in1=src[:, 1:5, :],
                                op=OP.subtract)     # nd = open - src
        tgt = u_nxt if k == 0 else A
        nc.vector.tensor_scalar(out=tgt[:], in0=B[:], scalar1=0.0,
                                scalar2=1.0, op0=OP.min, op1=OP.add)
        if k > 0:
            nc.vector.tensor_tensor(out=u_nxt[:], in0=A[:], in1=u_cur[:],
                                    op=OP.mult)


def build(k_t, d_p, d_t):
    nc = bacc.Bacc("TRN2", target_bir_lowering=False, debug=False,
                   num_devices=NCORES)
    pred_d = nc.dram_tensor("pred", [2 * 2, H, W], F32, kind="ExternalInput")
    targ_d = nc.dram_tensor("target", [2 * 2, H, W], I32, kind="ExternalInput")
    out_d = nc.dram_tensor("out", [2, P, STC], F32, kind="ExternalOutput")

    with TileContext(nc) as tc, \
            tc.tile_pool(name="main", bufs=1) as pool, \
            tc.tile_pool(name="psum", bufs=1, space="PSUM") as psum_pool:
        # ---- shift matrices for halo matmuls ----
        idx = pool.tile([P, 2, P], I32, name="idx", tag="idx")
        shm = pool.tile([P, 4, P], BF16, name="shm", tag="shm")
        nc.gpsimd.iota(idx[:, 0, :], pattern=[[1, P]], base=0,
                       channel_multiplier=-1)   # f - p
        nc.gpsimd.iota(idx[:, 1, :], pattern=[[1, P]], base=0,
                       channel_multiplier=1)    # f + p
        for j, (sl, val) in enumerate(((0, 1), (0, -1), (1, 0), (1, 254))):
            nc.vector.tensor_scalar(out=shm[:, j, :], in0=idx[:, sl, :],
                                    scalar1=val, scalar2=None, op0=OP.is_equal)

        bld = [_Pair(nc, pool, psum_pool, shm, p) for p in range(2)]

        # ---- input DMA (planar, with dtype cast); per-image chunks,
        # targets first, so the ACT/boundary chain starts ASAP ----
        for p, b in enumerate(bld):
            tv = targ_d[2 * p:2 * p + 2].rearrange("i (p r) c -> p i r c", p=P)
            for i in range(2):
                nc.gpsimd.dma_start(out=b.TB[:, i], in_=tv[:, i])  # i32->bf16
        for p, b in enumerate(bld):
            pv = pred_d[2 * p:2 * p + 2].rearrange("i (p r) c -> p i r c", p=P)
            for i in range(2):
                nc.gpsimd.dma_start(out=b.PRD[:, i], in_=pv[:, i])  # f32->bf16

        # ---- phase 1: pointwise stats + interleaved images (ACT-heavy) ----
        for b in bld:                     # copies first (Copy table)
            To = b.T[:, 1:5, :]
            for i in range(2):
                nc.scalar.activation(out=_img(To, i), in_=b.TB[:, i],
                                     func=AF.Copy,
                                     accum_out=b.ST[:, C_T + i:C_T + i + 1])
            for i in range(2):
                nc.scalar.activation(out=_img(b.LU[:], i), in_=b.PRD[:, i],
                                     func=AF.Copy)
        for b in bld:
            PRo = b.PR[:, 1:5, :]
            for i in range(2):
                nc.scalar.activation(out=_img(PRo, i), in_=b.PRD[:, i],
                                     func=AF.Sigmoid,
                                     accum_out=b.ST[:, C_P + i:C_P + i + 1])
            # softplus(x) = -ln(sigmoid(-x)); SP holds the NEGATED image
            # (host flips signs of C_SP / C_BSP)
            for i in range(2):
                nc.scalar.activation(out=_img(b.C[:], i), in_=b.PRD[:, i],
                                     func=AF.Sigmoid, scale=-1.0)
        for b in bld:
            for i in range(2):
                nc.scalar.activation(out=_img(b.SP[:], i), in_=_img(b.C[:], i),
                                     func=AF.Ln,
                                     accum_out=b.ST[:, C_SP + i:C_SP + i + 1])
        for b in bld:
            To = b.T[:, 1:5, :]
            # sum(pred*t), sum(prob*t), mask + sum(mask); scratch A/B
            nc.vector.tensor_tensor(out=b.A[:], in0=b.LU[:], in1=To,
                                    op=OP.mult)
            nc.scalar.activation(out=b.B[:], in_=b.A[:], func=AF.Copy,
                                 accum_out=b.ST[:, C_PT:C_PT + 1])
            nc.vector.tensor_tensor(out=b.C[:], in0=b.PR[:, 1:5, :], in1=To,
                                    op=OP.mult)
            nc.scalar.activation(out=b.B[:], in_=b.C[:], func=AF.Copy,
                                 accum_out=b.ST[:, C_PROBT:C_PROBT + 1])
            nc.vector.tensor_scalar(out=b.MK[:, 1:5, :], in0=b.LU[:],
                                    scalar1=0.0, scalar2=0.0, op0=OP.is_le,
                                    op1=OP.add,
                                    accum_out=b.ST[:, C_MASK:C_MASK + 1])
        for b in bld:
            b.halo(b.T)
            b.halo(b.PR)
            b.halo(b.MK)

        # ---- boundary loss sums ----
        for b in bld:
            nc_, To = b.nc, b.T[:, 1:5, :]
            b.make_e_tiles()  # PRD/TB dead from here (tag-shared memory)
            b.vpool(b.T, OP.max, b.B)
            b.hpool(b.B, OP.max, b.C, b.A)        # dilate -> C
            b.vpool(b.T, OP.min, b.B)
            b.hpool(b.B, OP.min, b.E1[:, 1:5, :], b.A)  # erode -> E1 owned
            nc_.vector.tensor_tensor(out=b.B[:], in0=b.C[:],
                                     in1=b.E1[:, 1:5, :],
                                     op=OP.subtract)          # b -> B
            nc_.vector.tensor_tensor(out=b.C[:], in0=b.B[:], in1=To,
                                     op=OP.mult)              # b*t -> C
            nc_.vector.tensor_tensor(out=b.A[:], in0=b.B[:], in1=b.SP[:],
                                     op=OP.mult)
            nc_.scalar.activation(out=b.E1[:, 1:5, :], in_=b.A[:],
                                  func=AF.Copy,
                                  accum_out=b.ST[:, C_BSP:C_BSP + 1])
            nc_.vector.tensor_tensor(out=b.B[:], in0=b.C[:], in1=b.LU[:],
                                     op=OP.mult)
            nc_.scalar.activation(out=b.E1[:, 1:5, :], in_=b.B[:],
                                  func=AF.Copy,
                                  accum_out=b.ST[:, C_BPT:C_BPT + 1])

        # ---- skeletons (pair-interleaved per iteration) ----
        def run_skeleton(src0_of, iters, w_of, col_prod, col_sum):
            srcs = {b: src0_of(b) for b in bld}
            ucur = {b: None for b in bld}
            for k in range(iters + 1):
                dsts = {b: (b.E1 if k % 2 == 0 else b.E2) for b in bld}
                for b in bld:
                    b.soft_erode5(srcs[b], dsts[b])
                for b in bld:
                    b.halo(dsts[b])
                for b in bld:
                    unxt = b.LU if k % 2 == 0 else b.SP
                    b.skel_dilate_update(srcs[b], dsts[b], k, ucur[b], unxt)
                    ucur[b] = unxt
                    srcs[b] = dsts[b]
            for b in bld:
                u = ucur[b]
                nc.vector.tensor_tensor(out=b.A[:], in0=u[:], in1=w_of(b),
                                        op=OP.mult)
                nc.scalar.activation(out=b.B[:], in_=b.A[:], func=AF.Copy,
                                     accum_out=b.ST[:, col_prod:col_prod + 1])
                nc.scalar.activation(out=b.C[:], in_=u[:], func=AF.Copy,
                                     accum_out=b.ST[:, col_sum:col_sum + 1])

        run_skeleton(lambda b: b.PR, K_SKEL, lambda b: b.T[:, 1:5, :],
                     C_SPT, C_SPS)
        run_skeleton(lambda b: b.T, k_t, lambda b: b.PR[:, 1:5, :],
                     C_STP, C_STS)

        # ---- distance transforms (pair-interleaved per iteration) ----
        def run_dt(m0_of, iters, finalize):
            srcs = {b: m0_of(b) for b in bld}
            accs = {b: None for b in bld}
            for d in range(1, iters + 1):
                dsts = {b: (b.E1 if d % 2 == 1 else b.E2) for b in bld}
                for b in bld:
                    b.erode3(srcs[b], dsts[b][:, 1:5, :])
                if d < iters:
                    for b in bld:
                        b.halo(dsts[b])
                for b in bld:
                    prev = (m0_of(b)[:, 1:5, :] if accs[b] is None
                            else accs[b][:])
                    acc_n = b.C if d % 2 == 1 else b.LU
                    nc.vector.tensor_add(out=acc_n[:], in0=prev,
                                         in1=dsts[b][:, 1:5, :])
                    accs[b] = acc_n
                    srcs[b] = dsts[b]
            for b in bld:
                final = (m0_of(b)[:, 1:5, :] if accs[b] is None
                         else accs[b][:])
                finalize(b, final)

        # DT of pred mask (MK), weighted by t
        def fin_p(b, final):
            nc.vector.tensor_tensor(out=b.B[:], in0=final,
                                    in1=b.T[:, 1:5, :], op=OP.mult)
            nc.scalar.activation(out=b.A[:], in_=b.B[:], func=AF.Copy,
                                 accum_out=b.ST[:, C_DTP:C_DTP + 1])
        run_dt(lambda b: b.MK, d_p, fin_p)

        # DT of (1 - t): overwrite T with its complement; weight = 1 - mask_p
        for b in bld:
            nc.vector.tensor_scalar(out=b.T[:, 1:5, :], in0=b.T[:, 1:5, :],
                                    scalar1=-1.0, scalar2=1.0, op0=OP.mult,
                                    op1=OP.add)
        for b in bld:
            b.halo(b.T)

        def fin_t(b, final):
            nc.scalar.activation(out=b.A[:], in_=final, func=AF.Copy,
                                 accum_out=b.ST[:, C_DTT:C_DTT + 1])
            nc.vector.tensor_tensor(out=b.B[:], in0=final,
                                    in1=b.MK[:, 1:5, :], op=OP.mult)
            nc.scalar.activation(out=b.C[:], in_=b.B[:], func=AF.Copy,
                                 accum_out=b.ST[:, C_DTTM:C_DTTM + 1])
        run_dt(lambda b: b.T, d_t, fin_t)

        for p, b in enumerate(bld):
            nc.sync.dma_start(out=out_d[p], in_=b.ST[:])
    nc.compile()
    return nc


# ---------------- host side ----------------
_cache = {}


def _bin_soft_erode(e):
    v = e & np.roll(e, 1, 1) & np.roll(e, -1, 1)
    v[:, 0] = e[:, 0] & e[:, 1]
    v[:, -1] = e[:, -1] & e[:, -2]
    h = e & np.roll(e, 1, 2) & np.roll(e, -1, 2)
    h[:, :, 0] = e[:, :, 0] & e[:, :, 1]
    h[:, :, -1] = e[:, :, -1] & e[:, :, -2]
    return v | h


def _bin_erode3(e):
    v = e & np.roll(e, 1, 1) & np.roll(e, -1, 1)
    v[:, 0] = e[:, 0] & e[:, 1]
    v[:, -1] = e[:, -1] & e[:, -2]
    h = v & np.roll(v, 1, 2) & np.roll(v, -1, 2)
    h[:, :, 0] = v[:, :, 0] & v[:, :, 1]
    h[:, :, -1] = v[:, :, -1] & v[:, :, -2]
    return h


def _needed_iters(mask, limit, erode_fn):
    """number of erosions until empty (capped)"""
    e, n = mask, 0
    while n < limit:
        e = erode_fn(e)
        if not e.any():
            break
        n += 1
    return n


def kernel(pred, target):
    pred = np.ascontiguousarray(np.asarray(pred), dtype=np.float32)
    target = np.ascontiguousarray(np.asarray(target), dtype=np.int32)
    B = pred.shape[0]
    p3 = pred.reshape(B, H, W)
    t3 = target.reshape(B, H, W)

    tb = t3 != 0
    k_t = _needed_iters(_bin_soft_erode(tb), K_SKEL - 1, _bin_soft_erode) + 1
    k_t = min(k_t, K_SKEL)
    d_p = _needed_iters(p3 <= 0.0, 19, _bin_erode3)
    d_t = _needed_iters(~tb, 19, _bin_erode3)

    key = (k_t, d_p, d_t)
    if key not in _cache:
        _cache[key] = build(*key)
    nc = _cache[key]

    in_maps = [
        {"pred": p3[4 * c:4 * c + 4], "target": t3[4 * c:4 * c + 4]}
        for c in range(NCORES)
    ]
    res = run_bass_kernel_spmd(nc, in_maps, core_ids=list(range(NCORES)))
    st = np.stack([r["out"] for r in res.results])  # [8, 2, 128, STC]
    s = st.sum(axis=(0, 1, 2), dtype=np.float64)    # summed stats

    N = float(pred.size)
    smooth, eps, hsm = 1.0, 1.0, 1e-6
    sum_sp = -(s[C_SP] + s[C_SP + 1])
    sum_pt = s[C_PT]
    sum_p = s[C_P] + s[C_P + 1]
    inter = s[C_PROBT]
    sum_t = s[C_T] + s[C_T + 1]
    loss_bce = (sum_sp - sum_pt) / N
    loss_dice = 1.0 - (2.0 * inter + smooth) / (sum_p + sum_t + smooth)
    fp = sum_p - inter
    fn = sum_t - inter
    tversky = (inter + smooth) / (inter + 0.3 * fp + 0.7 * fn + smooth)
    loss_ft = (1.0 - tversky) ** 1.33
    loss_boundary = loss_bce + 3.0 * (-s[C_BSP] - s[C_BPT]) / N
    tprec = ((sum_t - s[C_SPT]) + eps) / ((N - s[C_SPS]) + eps)
    tsens = ((sum_p - s[C_STP]) + eps) / ((N - s[C_STS]) + eps)
    loss_cldice = 1.0 - 2.0 * tprec * tsens / (tprec + tsens)
    dtp = s[C_DTP]
    dtt = s[C_DTT] - s[C_DTTM]
    n_pb = N - s[C_MASK]
    hd_fwd = (dtp + hsm) / (sum_t + hsm)
    hd_bwd = (dtt + hsm) / (n_pb + hsm)
    loss_hd = 0.5 * (hd_fwd + hd_bwd)
    total = (0.2 * loss_bce + 0.2 * loss_dice + 0.2 * loss_cldice
             + 0.1 * loss_hd + 0.1 * loss_boundary + 0.2 * loss_ft)
    return np.float32(total)
